# revision 34
# baseline (speedup 1.0000x reference)
"""Single-head causal attention on 8 TRN2 NeuronCores.

Problem: x [8, 2048, 1024] f32, Wq/Wk/Wv [1024, 64] f32.
  q = x @ Wq ; k = x @ Wk ; v = x @ Wv        (per batch)
  out = softmax(causal(q k^T / 8)) @ v        [8, 2048, 64]

Sharding: data-parallel over batch -- core i handles batch element i.
No collectives needed.

Per-core kernel (bf16 compute, f32 accumulate), 128-token-tile pipeline:
  1. W loads use the natural row-contiguous layout (2KB descriptors, no
     sub-512B DMA penalty); the d-contraction is chunked INTERLEAVED
     (chunk a = {d : d = 8p + a}) so the natural layout needs no
     on-chip weight transpose -- gpsimd packs [Wq|Wk|Wv] to bf16.
  2. x streams per 256-token group; each 128-tile is cast f32->bf16 on
     DVE directly into the interleaved layout, transposed on the PE
     (8x [128,128] identity matmuls -> PSUM bf16), and copied to the
     x^T SBUF pool by DVE.
  3. Projections are x-stationary: lhsT = x^T tile-chunk, moving
     rhs = [Wq|Wk|Wv] (192 wide) -> PSUM [t,192] in 8 matmuls/tile
     (1536 PE cycles vs 2048 for the W-stationary form).
  4. Q,K land t-major; one DMA-xbar transpose per tile ([t,128] ->
     [qk,t], 8 ucode tiles = ~112ns of DMA) yields Q^T/K^T rows with
     h on partitions. V stays t-major (what PV wants) and is copied
     into V_aug with a ones column (softmax denominator for free).
  5. Attention per 512-row q-block: S^T[tk,tq] = K^T_tile.T @ Q^T
     (contraction h); exp on ACT in k-tile PAIRS (halves the ~185ns
     per-instruction access-latency overhead); causal diagonal via a
     multiplicative 0/1 bf16 mask on DVE; PV accumulates
     out^T[65,tq] += V_aug.T @ P^T in PSUM, row 64 = denominators.
  6. Output: PSUM -> bf16 SBUF copy (gpsimd), PE-transpose back to
     [tq,65], reciprocal-rescale (DVE recip + gpsimd scale), bf16
     store (f32 upcast happens host-side after gather).

Engine budget: PE ~76k cycles (transposes 16.4k, proj 24.6k, S 16.9k,
PV 16.9k, out 1k) is the critical resource; DMA ~29us (x 23.3 =
roofline, W 2.2, qk-xbar 1.8, stores 1.5); ACT owns exp (~20us); DVE
casts/copies/masks (~22us); gpsimd does the PSUM drains (~11us).
"""

import numpy as np

import concourse.bass as bass
import concourse.tile as tile
from concourse import bacc, mybir
from concourse.bass_utils import run_bass_kernel_spmd

B, T, D, H = 8, 2048, 1024, 64
P = 128            # partitions / tile edge
ND = D // P        # 8 d-chunks (interleaved: chunk a = {d : d = 8p + a})
NT = T // P        # 16 token tiles
NB = T // 512      # 4 q-blocks of 512 rows
VA = 80            # v_aug padded k-tile stride

FP32 = mybir.dt.float32
BF16 = mybir.dt.bfloat16

_compiled = None


def _build():
    nc = bacc.Bacc("TRN2", target_bir_lowering=False, debug=False, num_devices=8)

    x_d = nc.dram_tensor("x", [T, D], FP32, kind="ExternalInput").ap()
    wq_d = nc.dram_tensor("Wq", [D, H], FP32, kind="ExternalInput").ap()
    wk_d = nc.dram_tensor("Wk", [D, H], FP32, kind="ExternalInput").ap()
    wv_d = nc.dram_tensor("Wv", [D, H], FP32, kind="ExternalInput").ap()
    out_d = nc.dram_tensor("out", [T, H], BF16, kind="ExternalOutput").ap()

    with tile.TileContext(nc) as tc:
        _kernel(tc, out_d, x_d, wq_d, wk_d, wv_d)

    nc.compile()
    return nc


def _kernel(tc, out_d, x_d, wq_d, wk_d, wv_d):
    nc = tc.nc
    from contextlib import ExitStack

    ctx = ExitStack()
    with ctx:
        const = ctx.enter_context(tc.tile_pool(name="const", bufs=1))
        wstage = ctx.enter_context(tc.tile_pool(name="wstage", bufs=3))
        xload = ctx.enter_context(tc.tile_pool(name="xload", bufs=6))
        xbtp = ctx.enter_context(tc.tile_pool(name="xbtp", bufs=4))
        xtp = ctx.enter_context(tc.tile_pool(name="xtp", bufs=1))
        qkp = ctx.enter_context(tc.tile_pool(name="qkp", bufs=1))
        qksp = ctx.enter_context(tc.tile_pool(name="qksp", bufs=2))
        vsb = ctx.enter_context(tc.tile_pool(name="vsb", bufs=1))
        ptp = ctx.enter_context(tc.tile_pool(name="ptp", bufs=3))
        obp = ctx.enter_context(tc.tile_pool(name="obp", bufs=2))
        osbp = ctx.enter_context(tc.tile_pool(name="osbp", bufs=2))
        recp = ctx.enter_context(tc.tile_pool(name="recp", bufs=2))
        pxt = ctx.enter_context(tc.tile_pool(name="pxt", bufs=1, space="PSUM"))
        psS = ctx.enter_context(tc.tile_pool(name="psS", bufs=2, space="PSUM"))
        pout = ctx.enter_context(tc.tile_pool(name="pout", bufs=1, space="PSUM"))
        psmall = ctx.enter_context(tc.tile_pool(name="psmall", bufs=2, space="PSUM"))

        # ---- loads: first x group, then weights, then remaining x ----
        # (x group 0 first so the cast/transpose pipeline starts ~2us
        # earlier; W only gates the projections, which queue behind.)
        x_r = x_d.rearrange("(g u p) d -> g p u d", p=P, u=2)
        xfs = {}

        def load_x(g):
            xf = xload.tile([P, 2, D], FP32, tag="xf", name=f"xf{g}")
            nc.sync.dma_start(out=xf[:], in_=x_r[g])
            xfs[g] = xf

        load_x(0)

        # Weight loads use the natural row-contiguous layout (2KB
        # descriptors; no sub-512B DMA penalty); chunk a of the interleaved
        # contraction is the partition-slice [:, a, :], so no weight
        # transpose is ever needed.
        w_all = const.tile([P, ND, 3 * H], BF16)   # [Wq | Wk | Wv] per slot
        wnats = []
        for w_dram, name in ((wq_d, "wq"), (wk_d, "wk"), (wv_d, "wv")):
            wn = wstage.tile([P, ND, H], FP32, tag="wstage", name=f"stg_{name}")
            nc.sync.dma_start(out=wn[:], in_=w_dram.rearrange(
                "(p a) h -> p a h", p=P))
            wnats.append(wn)
        for g in range(1, NT // 2):
            load_x(g)

        # Weight pack on ACT: its queue is empty until the first exps
        # (~12us), so waiting on the W loads here blocks nothing.
        for j, wn in enumerate(wnats):
            nc.scalar.copy(out=w_all[:, :, j * H:(j + 1) * H], in_=wn[:])

        # ---- constants ----
        ident_bf = const.tile([P, P], BF16)
        from concourse.masks import make_identity
        make_identity(nc, ident_bf[:])

        # 0/1 upper-triangular (incl. diagonal) bf16 mask in [tk, tq]
        # orientation: valid when tq >= tk (col >= row).
        tri01 = const.tile([P, P], BF16)
        nc.gpsimd.memset(tri01[:], 1.0)
        nc.gpsimd.affine_select(
            out=tri01[:], in_=tri01[:],
            compare_op=mybir.AluOpType.is_ge,
            fill=0.0, base=0,
            pattern=[[1, P]], channel_multiplier=-1)

        # Per-tile projection results, persistent: [:, i, 0:64] = Q,
        # [64:128] = K (both t-major, transposed later on the PE),
        # [128:192] = V, col 192 = 1.0 (softmax-denominator row for the
        # PV matmul's 65-row lhsT).  One DVE copy drains all three.
        vqk = vsb.tile([P, NT, 200], BF16)
        nc.gpsimd.memset(vqk[:, :, 3 * H:3 * H + 1], 1.0)


        # persistent SBUF state
        xT = xtp.tile([P, ND, T], BF16)      # x^T, interleaved chunks
        # Q^T/K^T ([:, 0] = Q^T, [:, 1] = K^T), h on partitions 0:64 --
        # produced by per-tile PE transposes (the DMA-xbar alternative
        # serializes against the x stream on the shared DMA engines and
        # poisons the in-order HWDGE queues with its waits).
        qkT = qkp.tile([H, 2, T], BF16)

        # ---- per-tile pipeline ----
        def tile_work(i):
            g, u = divmod(i, 2)
            # cast f32 -> bf16 into the interleaved (a, j) layout:
            # element d of the tile lands at [a = d % 8, j = d // 8].
            # Every third tile casts on the (otherwise idle) gpsimd so the
            # DVE keeps up with its PSUM-drain copies.
            xbt = xbtp.tile([P, ND, P], BF16, tag="xbt", name=f"xbt{i}")
            cast_eng = nc.gpsimd if i % 3 == 2 else nc.vector
            cast_eng.tensor_copy(
                out=xbt[:].rearrange("p a j -> p j a"), in_=xfs[g][:, u, :])
            # PE transposes: chunk a -> x^T[:, a, tile i]
            px = pxt.tile([P, ND, P], BF16, tag="pxt", name=f"px{i}")
            for a in range(ND):
                nc.tensor.transpose(px[:, a, :], xbt[:, a, :], ident_bf[:])
            nc.vector.tensor_copy(
                out=xT[:, :, i * P:(i + 1) * P], in_=px[:])
            # projection: x^T-stationary, W moving (192 wide)
            ps_p = psmall.tile([P, 3 * H], FP32, tag="small", name=f"psp{i}")
            for a in range(ND):
                nc.tensor.matmul(ps_p[:], xT[:, a, i * P:(i + 1) * P],
                                 w_all[:, a, :],
                                 start=(a == 0), stop=(a == ND - 1))
            # single drain: Q|K|V -> vqk (bf16)
            nc.vector.tensor_copy(out=vqk[:, i, 0:3 * H], in_=ps_p[:])
            # Q,K -> PE transposes -> qkT
            pqk = psmall.tile([H, 2, P], BF16, tag="small", name=f"pqk{i}")
            for u in range(2):
                nc.tensor.transpose(pqk[:, u, :], vqk[:, i, u * H:(u + 1) * H],
                                    ident_bf[:])
            nc.vector.tensor_copy(out=qkT[:, :, i * P:(i + 1) * P], in_=pqk[:])

        # ---- attention ----
        stores = []

        def diag(b, ki):
            return 4 * b <= ki < 4 * b + 4

        def attention_block(b):
            nk = 4 * b + 4
            qlo = 512 * b
            pairs = [(2 * j, 2 * j + 1) for j in range(nk // 2)]
            ps_o = pout.tile([H + 1, 512], FP32, tag="pout", name=f"pso{b}")

            def s_exp(pr):
                k0, k1 = pr
                w0 = max(0, k0 * P - qlo)
                w1 = max(0, k1 * P - qlo)
                ps = psS.tile([P, 1024], FP32, tag="psS", name=f"psS{b}_{k0}")
                pt = ptp.tile([P, 1024], BF16, tag="pt", name=f"pt{b}_{k0}")
                for ki, w, pos in ((k0, w0, 0), (k1, w1, 512)):
                    nc.tensor.matmul(
                        ps[:, pos + w:pos + 512],
                        qkT[:, 1, ki * P:(ki + 1) * P],
                        qkT[:, 0, qlo + w:qlo + 512],
                        start=True, stop=True)
                if b == 0:
                    # fresh PSUM slots: exp only over written regions
                    for ki, w, pos in ((k0, w0, 0), (k1, w1, 512)):
                        nc.scalar.activation(
                            out=pt[:, pos + w:pos + 512],
                            in_=ps[:, pos + w:pos + 512],
                            func=mybir.ActivationFunctionType.Exp,
                            scale=0.125)
                else:
                    # one wide exp; the [512, 512+w1) gap holds stale
                    # (finite) values from an earlier pair and is never
                    # read by PV.
                    nc.scalar.activation(
                        out=pt[:, w0:1024], in_=ps[:, w0:1024],
                        func=mybir.ActivationFunctionType.Exp,
                        scale=0.125)
                for ki, w, pos in ((k0, w0, 0), (k1, w1, 512)):
                    if diag(b, ki):
                        nc.gpsimd.tensor_mul(pt[:, pos + w:pos + w + P],
                                             pt[:, pos + w:pos + w + P],
                                             tri01[:])
                return pt, w0, w1

            def pv(idx, pr, pt_w):
                k0, k1 = pr
                pt, w0, w1 = pt_w
                for ki, w, pos in ((k0, w0, 0), (k1, w1, 512)):
                    nc.tensor.matmul(
                        ps_o[:, w:512], vqk[:, ki, 2 * H:3 * H + 1],
                        pt[:, pos + w:pos + 512],
                        start=(idx == 0 and ki == k0),
                        stop=(idx == len(pairs) - 1 and ki == k1))

            pending = s_exp(pairs[0])
            for idx, pr in enumerate(pairs):
                nxt = s_exp(pairs[idx + 1]) if idx + 1 < len(pairs) else None
                pv(idx, pr, pending)
                pending = nxt
            # Drain PSUM inline (frees the single pout slot for the next
            # block; also ACT-queue position right after this block's exps
            # keeps later blocks' PV from serializing behind it).
            ob = obp.tile([H + 1, 512], BF16, tag="ob", name=f"ob{b}")
            nc.scalar.copy(out=ob[:], in_=ps_o[:])
            return ob

        def out_stage(b, ob):
            # Deferred to the tail: these waits must never head-of-line
            # block streaming work.
            pot = psmall.tile([P, 4, VA], BF16, tag="small", name=f"pot{b}")
            for j in range(4):
                nc.tensor.transpose(pot[:, j, 0:H + 1],
                                    ob[:, j * P:(j + 1) * P],
                                    ident_bf[0:H + 1, 0:H + 1])
            # Only TWO DVE queue entries here (copy + recip) -- they park in
            # the 4-deep wait queue without stalling DVE.SEQ for the
            # streaming casts behind them; the rescales run on idle gpsimd
            # from SBUF.
            ot = osbp.tile([P, 4, H + 1], BF16, tag="ot", name=f"ot{b}")
            nc.vector.tensor_copy(out=ot[:], in_=pot[:, :, 0:H + 1])
            rec = recp.tile([P, 4], FP32, tag="rec", name=f"rec{b}")
            nc.vector.reciprocal(rec[:], ot[:, :, H])
            osb = osbp.tile([P, 4, H], BF16, tag="osb", name=f"osb{b}")
            for j in range(4):
                nc.gpsimd.tensor_scalar_mul(osb[:, j, :], ot[:, j, 0:H],
                                            rec[:, j:j + 1])
            stores.append(
                (out_d.rearrange("(b j p) h -> b p j h", p=P, j=4)[b], osb))

        # Emission tracks data arrival: block b's S-work is emitted ~2
        # tiles after its last q-tile (when its Q^T/K^T exist) so parked
        # PE instructions never head-of-line block ready ones; everything
        # past the PV accumulate (PE re-transpose, reciprocal rescale,
        # stores) is deferred to the tail so its waits never block
        # streaming work on the in-order queues.
        obs = {}
        for i in range(NT):
            tile_work(i)
            if i >= 5 and (i - 5) % 4 == 0:
                b = (i - 5) // 4
                obs[b] = attention_block(b)
            if i >= 9 and (i - 9) % 4 == 0:
                b = (i - 9) // 4
                out_stage(b, obs.pop(b))
        obs[3] = attention_block(3)
        out_stage(2, obs.pop(2))
        out_stage(3, obs.pop(3))

        for dst, osb in stores:
            nc.sync.dma_start(out=dst, in_=osb[:])


def _run(inputs, trace=False, **kw):
    global _compiled
    if _compiled is None:
        _compiled = _build()
    nc = _compiled
    x = np.ascontiguousarray(inputs["x"], dtype=np.float32)
    wq = np.ascontiguousarray(inputs["Wq"], dtype=np.float32)
    wk = np.ascontiguousarray(inputs["Wk"], dtype=np.float32)
    wv = np.ascontiguousarray(inputs["Wv"], dtype=np.float32)
    in_maps = [
        {"x": np.ascontiguousarray(x[i]), "Wq": wq, "Wk": wk, "Wv": wv}
        for i in range(B)
    ]
    res = run_bass_kernel_spmd(nc, in_maps, core_ids=list(range(B)),
                               trace=trace, **kw)
    out = np.stack(
        [np.asarray(res.results[i]["out"]).astype(np.float32) for i in range(B)],
        axis=0)
    return out, res


def kernel(x, Wq, Wk, Wv):
    out, _ = _run({"x": x, "Wq": Wq, "Wk": Wk, "Wv": Wv})
    return out


# revision 35
# speedup vs baseline: 1.0020x; 1.0020x over previous
"""Single-head causal attention on 8 TRN2 NeuronCores.

Problem: x [8, 2048, 1024] f32, Wq/Wk/Wv [1024, 64] f32.
  q = x @ Wq ; k = x @ Wk ; v = x @ Wv        (per batch)
  out = softmax(causal(q k^T / 8)) @ v        [8, 2048, 64]

Sharding: data-parallel over batch -- core i handles batch element i.
No collectives needed.

Per-core kernel (bf16 compute, f32 accumulate), 128-token-tile pipeline:
  1. W loads use the natural row-contiguous layout (2KB descriptors, no
     sub-512B DMA penalty); the d-contraction is chunked INTERLEAVED
     (chunk a = {d : d = 8p + a}) so the natural layout needs no
     on-chip weight transpose -- gpsimd packs [Wq|Wk|Wv] to bf16.
  2. x streams per 256-token group; each 128-tile is cast f32->bf16 on
     DVE directly into the interleaved layout, transposed on the PE
     (8x [128,128] identity matmuls -> PSUM bf16), and copied to the
     x^T SBUF pool by DVE.
  3. Projections are x-stationary: lhsT = x^T tile-chunk, moving
     rhs = [Wq|Wk|Wv] (192 wide) -> PSUM [t,192] in 8 matmuls/tile
     (1536 PE cycles vs 2048 for the W-stationary form).
  4. Q,K land t-major; one DMA-xbar transpose per tile ([t,128] ->
     [qk,t], 8 ucode tiles = ~112ns of DMA) yields Q^T/K^T rows with
     h on partitions. V stays t-major (what PV wants) and is copied
     into V_aug with a ones column (softmax denominator for free).
  5. Attention per 512-row q-block: S^T[tk,tq] = K^T_tile.T @ Q^T
     (contraction h); exp on ACT in k-tile PAIRS (halves the ~185ns
     per-instruction access-latency overhead); causal diagonal via a
     multiplicative 0/1 bf16 mask on DVE; PV accumulates
     out^T[65,tq] += V_aug.T @ P^T in PSUM, row 64 = denominators.
  6. Output: PSUM -> bf16 SBUF copy (gpsimd), PE-transpose back to
     [tq,65], reciprocal-rescale (DVE recip + gpsimd scale), bf16
     store (f32 upcast happens host-side after gather).

Engine budget: PE ~76k cycles (transposes 16.4k, proj 24.6k, S 16.9k,
PV 16.9k, out 1k) is the critical resource; DMA ~29us (x 23.3 =
roofline, W 2.2, qk-xbar 1.8, stores 1.5); ACT owns exp (~20us); DVE
casts/copies/masks (~22us); gpsimd does the PSUM drains (~11us).
"""

import numpy as np

import concourse.bass as bass
import concourse.tile as tile
from concourse import bacc, mybir
from concourse.bass_utils import run_bass_kernel_spmd

B, T, D, H = 8, 2048, 1024, 64
P = 128            # partitions / tile edge
ND = D // P        # 8 d-chunks (interleaved: chunk a = {d : d = 8p + a})
NT = T // P        # 16 token tiles
NB = T // 512      # 4 q-blocks of 512 rows
VA = 80            # v_aug padded k-tile stride

FP32 = mybir.dt.float32
BF16 = mybir.dt.bfloat16

_compiled = None


def _build():
    nc = bacc.Bacc("TRN2", target_bir_lowering=False, debug=False, num_devices=8)

    x_d = nc.dram_tensor("x", [T, D], FP32, kind="ExternalInput").ap()
    wq_d = nc.dram_tensor("Wq", [D, H], FP32, kind="ExternalInput").ap()
    wk_d = nc.dram_tensor("Wk", [D, H], FP32, kind="ExternalInput").ap()
    wv_d = nc.dram_tensor("Wv", [D, H], FP32, kind="ExternalInput").ap()
    out_d = nc.dram_tensor("out", [T, H], BF16, kind="ExternalOutput").ap()

    with tile.TileContext(nc) as tc:
        _kernel(tc, out_d, x_d, wq_d, wk_d, wv_d)

    nc.compile()
    return nc


def _kernel(tc, out_d, x_d, wq_d, wk_d, wv_d):
    nc = tc.nc
    from contextlib import ExitStack

    ctx = ExitStack()
    with ctx:
        const = ctx.enter_context(tc.tile_pool(name="const", bufs=1))
        wstage = ctx.enter_context(tc.tile_pool(name="wstage", bufs=3))
        xload = ctx.enter_context(tc.tile_pool(name="xload", bufs=6))
        xbtp = ctx.enter_context(tc.tile_pool(name="xbtp", bufs=4))
        xtp = ctx.enter_context(tc.tile_pool(name="xtp", bufs=1))
        qkp = ctx.enter_context(tc.tile_pool(name="qkp", bufs=1))
        qksp = ctx.enter_context(tc.tile_pool(name="qksp", bufs=2))
        vsb = ctx.enter_context(tc.tile_pool(name="vsb", bufs=1))
        ptp = ctx.enter_context(tc.tile_pool(name="ptp", bufs=3))
        obp = ctx.enter_context(tc.tile_pool(name="obp", bufs=2))
        osbp = ctx.enter_context(tc.tile_pool(name="osbp", bufs=2))
        recp = ctx.enter_context(tc.tile_pool(name="recp", bufs=2))
        pxt = ctx.enter_context(tc.tile_pool(name="pxt", bufs=1, space="PSUM"))
        psS = ctx.enter_context(tc.tile_pool(name="psS", bufs=2, space="PSUM"))
        pout = ctx.enter_context(tc.tile_pool(name="pout", bufs=1, space="PSUM"))
        psmall = ctx.enter_context(tc.tile_pool(name="psmall", bufs=2, space="PSUM"))

        # ---- loads: first x group, then weights, then remaining x ----
        # (x group 0 first so the cast/transpose pipeline starts ~2us
        # earlier; W only gates the projections, which queue behind.)
        x_r = x_d.rearrange("(g u p) d -> g p u d", p=P, u=2)
        xfs = {}

        def load_x(g):
            xf = xload.tile([P, 2, D], FP32, tag="xf", name=f"xf{g}")
            nc.sync.dma_start(out=xf[:], in_=x_r[g])
            xfs[g] = xf

        load_x(0)

        # Weight loads use the natural row-contiguous layout (2KB
        # descriptors; no sub-512B DMA penalty); chunk a of the interleaved
        # contraction is the partition-slice [:, a, :], so no weight
        # transpose is ever needed.
        w_all = const.tile([P, ND, 3 * H], BF16)   # [Wq | Wk | Wv] per slot
        wnats = []
        for w_dram, name in ((wq_d, "wq"), (wk_d, "wk"), (wv_d, "wv")):
            wn = wstage.tile([P, ND, H], FP32, tag="wstage", name=f"stg_{name}")
            nc.sync.dma_start(out=wn[:], in_=w_dram.rearrange(
                "(p a) h -> p a h", p=P))
            wnats.append(wn)
        for g in range(1, NT // 2):
            load_x(g)

        # Weight pack on ACT: its queue is empty until the first exps
        # (~12us), so waiting on the W loads here blocks nothing.
        for j, wn in enumerate(wnats):
            nc.scalar.copy(out=w_all[:, :, j * H:(j + 1) * H], in_=wn[:])

        # ---- constants ----
        ident_bf = const.tile([P, P], BF16)
        from concourse.masks import make_identity
        make_identity(nc, ident_bf[:])

        # 0/1 upper-triangular (incl. diagonal) bf16 mask in [tk, tq]
        # orientation: valid when tq >= tk (col >= row).
        tri01 = const.tile([P, P], BF16)
        nc.gpsimd.memset(tri01[:], 1.0)
        nc.gpsimd.affine_select(
            out=tri01[:], in_=tri01[:],
            compare_op=mybir.AluOpType.is_ge,
            fill=0.0, base=0,
            pattern=[[1, P]], channel_multiplier=-1)

        # Per-tile projection results, persistent: [:, i, 0:64] = Q,
        # [64:128] = K (both t-major, transposed later on the PE),
        # [128:192] = V, col 192 = 1.0 (softmax-denominator row for the
        # PV matmul's 65-row lhsT).  One DVE copy drains all three.
        vqk = vsb.tile([P, NT, 200], BF16)
        nc.gpsimd.memset(vqk[:, :, 3 * H:3 * H + 1], 1.0)


        # persistent SBUF state
        xT = xtp.tile([P, ND, T], BF16)      # x^T, interleaved chunks
        # Q^T/K^T ([:, 0] = Q^T, [:, 1] = K^T), h on partitions 0:64 --
        # produced by per-tile PE transposes (the DMA-xbar alternative
        # serializes against the x stream on the shared DMA engines and
        # poisons the in-order HWDGE queues with its waits).
        qkT = qkp.tile([H, 2, T], BF16)

        # ---- per-tile pipeline ----
        def tile_work(i):
            g, u = divmod(i, 2)
            # cast f32 -> bf16 into the interleaved (a, j) layout:
            # element d of the tile lands at [a = d % 8, j = d // 8].
            # Every third tile casts on the (otherwise idle) gpsimd so the
            # DVE keeps up with its PSUM-drain copies.
            xbt = xbtp.tile([P, ND, P], BF16, tag="xbt", name=f"xbt{i}")
            cast_eng = nc.gpsimd if i % 3 == 2 else nc.vector
            cast_eng.tensor_copy(
                out=xbt[:].rearrange("p a j -> p j a"), in_=xfs[g][:, u, :])
            # PE transposes: chunk a -> x^T[:, a, tile i]
            px = pxt.tile([P, ND, P], BF16, tag="pxt", name=f"px{i}")
            for a in range(ND):
                nc.tensor.transpose(px[:, a, :], xbt[:, a, :], ident_bf[:])
            nc.vector.tensor_copy(
                out=xT[:, :, i * P:(i + 1) * P], in_=px[:])
            # projection: x^T-stationary, W moving (192 wide)
            ps_p = psmall.tile([P, 3 * H], FP32, tag="small", name=f"psp{i}")
            for a in range(ND):
                nc.tensor.matmul(ps_p[:], xT[:, a, i * P:(i + 1) * P],
                                 w_all[:, a, :],
                                 start=(a == 0), stop=(a == ND - 1))
            # single drain: Q|K|V -> vqk (bf16)
            nc.vector.tensor_copy(out=vqk[:, i, 0:3 * H], in_=ps_p[:])
            # Q,K -> PE transposes -> qkT
            pqk = psmall.tile([H, 2, P], BF16, tag="small", name=f"pqk{i}")
            for u in range(2):
                nc.tensor.transpose(pqk[:, u, :], vqk[:, i, u * H:(u + 1) * H],
                                    ident_bf[:])
            nc.vector.tensor_copy(out=qkT[:, :, i * P:(i + 1) * P], in_=pqk[:])

        # ---- attention ----
        stores = []

        def diag(b, ki):
            return 4 * b <= ki < 4 * b + 4

        def attention_block(b):
            nk = 4 * b + 4
            qlo = 512 * b
            pairs = [(2 * j, 2 * j + 1) for j in range(nk // 2)]
            ps_o = pout.tile([H + 1, 512], FP32, tag="pout", name=f"pso{b}")

            def s_exp(pr):
                k0, k1 = pr
                w0 = max(0, k0 * P - qlo)
                w1 = max(0, k1 * P - qlo)
                ps = psS.tile([P, 1024], FP32, tag="psS", name=f"psS{b}_{k0}")
                pt = ptp.tile([P, 1024], BF16, tag="pt", name=f"pt{b}_{k0}")
                for ki, w, pos in ((k0, w0, 0), (k1, w1, 512)):
                    nc.tensor.matmul(
                        ps[:, pos + w:pos + 512],
                        qkT[:, 1, ki * P:(ki + 1) * P],
                        qkT[:, 0, qlo + w:qlo + 512],
                        start=True, stop=True)
                if b == 0:
                    # fresh PSUM slots: exp only over written regions
                    for ki, w, pos in ((k0, w0, 0), (k1, w1, 512)):
                        nc.scalar.activation(
                            out=pt[:, pos + w:pos + 512],
                            in_=ps[:, pos + w:pos + 512],
                            func=mybir.ActivationFunctionType.Exp,
                            scale=0.125)
                else:
                    # one wide exp; the [512, 512+w1) gap holds stale
                    # (finite) values from an earlier pair and is never
                    # read by PV.
                    nc.scalar.activation(
                        out=pt[:, w0:1024], in_=ps[:, w0:1024],
                        func=mybir.ActivationFunctionType.Exp,
                        scale=0.125)
                for ki, w, pos in ((k0, w0, 0), (k1, w1, 512)):
                    if diag(b, ki):
                        nc.gpsimd.tensor_mul(pt[:, pos + w:pos + w + P],
                                             pt[:, pos + w:pos + w + P],
                                             tri01[:])
                return pt, w0, w1

            def pv(idx, pr, pt_w):
                k0, k1 = pr
                pt, w0, w1 = pt_w
                for ki, w, pos in ((k0, w0, 0), (k1, w1, 512)):
                    nc.tensor.matmul(
                        ps_o[:, w:512], vqk[:, ki, 2 * H:3 * H + 1],
                        pt[:, pos + w:pos + 512],
                        start=(idx == 0 and ki == k0),
                        stop=(idx == len(pairs) - 1 and ki == k1))

            pending = s_exp(pairs[0])
            for idx, pr in enumerate(pairs):
                nxt = s_exp(pairs[idx + 1]) if idx + 1 < len(pairs) else None
                pv(idx, pr, pending)
                pending = nxt
            # Drain PSUM inline (frees the single pout slot for the next
            # block; also ACT-queue position right after this block's exps
            # keeps later blocks' PV from serializing behind it).
            ob = obp.tile([H + 1, 512], BF16, tag="ob", name=f"ob{b}")
            nc.scalar.copy(out=ob[:], in_=ps_o[:])
            return ob

        def out_stage(b, ob):
            # Deferred to the tail: these waits must never head-of-line
            # block streaming work.
            pot = psmall.tile([P, 4, VA], BF16, tag="small", name=f"pot{b}")
            for j in range(4):
                nc.tensor.transpose(pot[:, j, 0:H + 1],
                                    ob[:, j * P:(j + 1) * P],
                                    ident_bf[0:H + 1, 0:H + 1])
            # Only TWO DVE queue entries here (copy + recip) -- they park in
            # the 4-deep wait queue without stalling DVE.SEQ for the
            # streaming casts behind them; the rescales run on idle gpsimd
            # from SBUF.
            ot = osbp.tile([P, 4, H + 1], BF16, tag="ot", name=f"ot{b}")
            nc.vector.tensor_copy(out=ot[:], in_=pot[:, :, 0:H + 1])
            rec = recp.tile([P, 4], FP32, tag="rec", name=f"rec{b}")
            nc.vector.reciprocal(rec[:], ot[:, :, H])
            osb = osbp.tile([P, 4, H], BF16, tag="osb", name=f"osb{b}")
            for j in range(4):
                nc.gpsimd.tensor_scalar_mul(osb[:, j, :], ot[:, j, 0:H],
                                            rec[:, j:j + 1])
            stores.append(
                (out_d.rearrange("(b j p) h -> b p j h", p=P, j=4)[b], osb))

        # Emission tracks data arrival: block b's S-work is emitted ~2
        # tiles after its last q-tile (when its Q^T/K^T exist) so parked
        # PE instructions never head-of-line block ready ones; everything
        # past the PV accumulate (PE re-transpose, reciprocal rescale,
        # stores) is deferred to the tail so its waits never block
        # streaming work on the in-order queues.
        obs = {}
        for i in range(NT):
            tile_work(i)
            # stage b-1 BEFORE block b+1: its PE transposes must sit ahead
            # of the next block's S/PV stream in the in-order PE queue, or
            # the DVE-side ot-copy waits out the whole block and starves
            # the remaining tiles' copies.
            if i >= 9 and (i - 9) % 4 == 0:
                b = (i - 9) // 4
                out_stage(b, obs.pop(b))
            if i >= 5 and (i - 5) % 4 == 0:
                b = (i - 5) // 4
                obs[b] = attention_block(b)
        out_stage(2, obs.pop(2))
        obs[3] = attention_block(3)
        out_stage(3, obs.pop(3))

        for dst, osb in stores:
            nc.sync.dma_start(out=dst, in_=osb[:])


def _run(inputs, trace=False, **kw):
    global _compiled
    if _compiled is None:
        _compiled = _build()
    nc = _compiled
    x = np.ascontiguousarray(inputs["x"], dtype=np.float32)
    wq = np.ascontiguousarray(inputs["Wq"], dtype=np.float32)
    wk = np.ascontiguousarray(inputs["Wk"], dtype=np.float32)
    wv = np.ascontiguousarray(inputs["Wv"], dtype=np.float32)
    in_maps = [
        {"x": np.ascontiguousarray(x[i]), "Wq": wq, "Wk": wk, "Wv": wv}
        for i in range(B)
    ]
    res = run_bass_kernel_spmd(nc, in_maps, core_ids=list(range(B)),
                               trace=trace, **kw)
    out = np.stack(
        [np.asarray(res.results[i]["out"]).astype(np.float32) for i in range(B)],
        axis=0)
    return out, res


def kernel(x, Wq, Wk, Wv):
    out, _ = _run({"x": x, "Wq": Wq, "Wk": Wk, "Wv": Wv})
    return out


# revision 36
# speedup vs baseline: 1.0268x; 1.0248x over previous
"""Single-head causal attention on 8 TRN2 NeuronCores.

Problem: x [8, 2048, 1024] f32, Wq/Wk/Wv [1024, 64] f32.
  q = x @ Wq ; k = x @ Wk ; v = x @ Wv        (per batch)
  out = softmax(causal(q k^T / 8)) @ v        [8, 2048, 64]

Sharding: data-parallel over batch -- core i handles batch element i.
No collectives needed.

Per-core kernel (bf16 compute, f32 accumulate), 128-token-tile pipeline:
  1. W loads use the natural row-contiguous layout (2KB descriptors, no
     sub-512B DMA penalty); the d-contraction is chunked INTERLEAVED
     (chunk a = {d : d = 8p + a}) so the natural layout needs no
     on-chip weight transpose -- gpsimd packs [Wq|Wk|Wv] to bf16.
  2. x streams per 256-token group; each 128-tile is cast f32->bf16 on
     DVE directly into the interleaved layout, transposed on the PE
     (8x [128,128] identity matmuls -> PSUM bf16), and copied to the
     x^T SBUF pool by DVE.
  3. Projections are x-stationary: lhsT = x^T tile-chunk, moving
     rhs = [Wq|Wk|Wv] (192 wide) -> PSUM [t,192] in 8 matmuls/tile
     (1536 PE cycles vs 2048 for the W-stationary form).
  4. Q,K land t-major; one DMA-xbar transpose per tile ([t,128] ->
     [qk,t], 8 ucode tiles = ~112ns of DMA) yields Q^T/K^T rows with
     h on partitions. V stays t-major (what PV wants) and is copied
     into V_aug with a ones column (softmax denominator for free).
  5. Attention per 512-row q-block: S^T[tk,tq] = K^T_tile.T @ Q^T
     (contraction h); exp on ACT in k-tile PAIRS (halves the ~185ns
     per-instruction access-latency overhead); causal diagonal via a
     multiplicative 0/1 bf16 mask on DVE; PV accumulates
     out^T[65,tq] += V_aug.T @ P^T in PSUM, row 64 = denominators.
  6. Output: PSUM -> bf16 SBUF copy (gpsimd), PE-transpose back to
     [tq,65], reciprocal-rescale (DVE recip + gpsimd scale), bf16
     store (f32 upcast happens host-side after gather).

Engine budget: PE ~76k cycles (transposes 16.4k, proj 24.6k, S 16.9k,
PV 16.9k, out 1k) is the critical resource; DMA ~29us (x 23.3 =
roofline, W 2.2, qk-xbar 1.8, stores 1.5); ACT owns exp (~20us); DVE
casts/copies/masks (~22us); gpsimd does the PSUM drains (~11us).
"""

import numpy as np

import concourse.bass as bass
import concourse.tile as tile
from concourse import bacc, mybir
from concourse.bass_utils import run_bass_kernel_spmd

B, T, D, H = 8, 2048, 1024, 64
P = 128            # partitions / tile edge
ND = D // P        # 8 d-chunks (interleaved: chunk a = {d : d = 8p + a})
NT = T // P        # 16 token tiles
NB = T // 512      # 4 q-blocks of 512 rows
VA = 80            # v_aug padded k-tile stride

FP32 = mybir.dt.float32
BF16 = mybir.dt.bfloat16

_compiled = None


def _build():
    nc = bacc.Bacc("TRN2", target_bir_lowering=False, debug=False, num_devices=8)

    x_d = nc.dram_tensor("x", [T, D], FP32, kind="ExternalInput").ap()
    wq_d = nc.dram_tensor("Wq", [D, H], FP32, kind="ExternalInput").ap()
    wk_d = nc.dram_tensor("Wk", [D, H], FP32, kind="ExternalInput").ap()
    wv_d = nc.dram_tensor("Wv", [D, H], FP32, kind="ExternalInput").ap()
    out_d = nc.dram_tensor("out", [T, H], BF16, kind="ExternalOutput").ap()

    with tile.TileContext(nc) as tc:
        _kernel(tc, out_d, x_d, wq_d, wk_d, wv_d)

    nc.compile()
    return nc


def _kernel(tc, out_d, x_d, wq_d, wk_d, wv_d):
    nc = tc.nc
    from contextlib import ExitStack

    ctx = ExitStack()
    with ctx:
        const = ctx.enter_context(tc.tile_pool(name="const", bufs=1))
        wstage = ctx.enter_context(tc.tile_pool(name="wstage", bufs=3))
        xload = ctx.enter_context(tc.tile_pool(name="xload", bufs=6))
        xbtp = ctx.enter_context(tc.tile_pool(name="xbtp", bufs=4))
        xtp = ctx.enter_context(tc.tile_pool(name="xtp", bufs=1))
        qkp = ctx.enter_context(tc.tile_pool(name="qkp", bufs=1))
        qksp = ctx.enter_context(tc.tile_pool(name="qksp", bufs=2))
        vsb = ctx.enter_context(tc.tile_pool(name="vsb", bufs=1))
        ptp = ctx.enter_context(tc.tile_pool(name="ptp", bufs=3))
        obp = ctx.enter_context(tc.tile_pool(name="obp", bufs=2))
        osbp = ctx.enter_context(tc.tile_pool(name="osbp", bufs=2))
        recp = ctx.enter_context(tc.tile_pool(name="recp", bufs=2))
        pxt = ctx.enter_context(tc.tile_pool(name="pxt", bufs=1, space="PSUM"))
        psS = ctx.enter_context(tc.tile_pool(name="psS", bufs=2, space="PSUM"))
        pout = ctx.enter_context(tc.tile_pool(name="pout", bufs=1, space="PSUM"))
        psmall = ctx.enter_context(tc.tile_pool(name="psmall", bufs=2, space="PSUM"))

        # ---- loads: first x group, then weights, then remaining x ----
        # (x group 0 first so the cast/transpose pipeline starts ~2us
        # earlier; W only gates the projections, which queue behind.)
        x_r = x_d.rearrange("(g u p) d -> g p u d", p=P, u=2)
        xfs = {}

        def load_x(g):
            xf = xload.tile([P, 2, D], FP32, tag="xf", name=f"xf{g}")
            nc.sync.dma_start(out=xf[:], in_=x_r[g])
            xfs[g] = xf

        load_x(0)

        # Weight loads use the natural row-contiguous layout (2KB
        # descriptors; no sub-512B DMA penalty); chunk a of the interleaved
        # contraction is the partition-slice [:, a, :], so no weight
        # transpose is ever needed.
        w_all = const.tile([P, ND, 3 * H], BF16)   # [Wq | Wk | Wv] per slot
        wnats = []
        for w_dram, name in ((wq_d, "wq"), (wk_d, "wk"), (wv_d, "wv")):
            wn = wstage.tile([P, ND, H], FP32, tag="wstage", name=f"stg_{name}")
            nc.sync.dma_start(out=wn[:], in_=w_dram.rearrange(
                "(p a) h -> p a h", p=P))
            wnats.append(wn)
        for g in range(1, NT // 2):
            load_x(g)

        # Weight pack on ACT: its queue is empty until the first exps
        # (~12us), so waiting on the W loads here blocks nothing.
        for j, wn in enumerate(wnats):
            nc.scalar.copy(out=w_all[:, :, j * H:(j + 1) * H], in_=wn[:])

        # ---- constants ----
        ident_bf = const.tile([P, P], BF16)
        from concourse.masks import make_identity
        make_identity(nc, ident_bf[:])

        # 0/1 upper-triangular (incl. diagonal) bf16 mask in [tk, tq]
        # orientation: valid when tq >= tk (col >= row).
        tri01 = const.tile([P, P], BF16)
        nc.gpsimd.memset(tri01[:], 1.0)
        nc.gpsimd.affine_select(
            out=tri01[:], in_=tri01[:],
            compare_op=mybir.AluOpType.is_ge,
            fill=0.0, base=0,
            pattern=[[1, P]], channel_multiplier=-1)

        # Per-tile projection results, persistent: [:, i, 0:64] = Q,
        # [64:128] = K (both t-major, transposed later on the PE),
        # [128:192] = V, col 192 = 1.0 (softmax-denominator row for the
        # PV matmul's 65-row lhsT).  One DVE copy drains all three.
        vqk = vsb.tile([P, NT, 200], BF16)
        nc.gpsimd.memset(vqk[:, :, 3 * H:3 * H + 1], 1.0)


        # persistent SBUF state
        xT = xtp.tile([P, ND, T], BF16)      # x^T, interleaved chunks
        # Q^T/K^T ([:, 0] = Q^T, [:, 1] = K^T), h on partitions 0:64 --
        # produced by per-tile PE transposes (the DMA-xbar alternative
        # serializes against the x stream on the shared DMA engines and
        # poisons the in-order HWDGE queues with its waits).
        qkT = qkp.tile([H, 2, T], BF16)

        # ---- per-tile pipeline ----
        def tile_work(i):
            g, u = divmod(i, 2)
            # cast f32 -> bf16 into the interleaved (a, j) layout:
            # element d of the tile lands at [a = d % 8, j = d // 8].
            # Every third tile casts on the (otherwise idle) gpsimd so the
            # DVE keeps up with its PSUM-drain copies.
            xbt = xbtp.tile([P, ND, P], BF16, tag="xbt", name=f"xbt{i}")
            cast_eng = nc.gpsimd if i % 3 == 2 else nc.vector
            cast_eng.tensor_copy(
                out=xbt[:].rearrange("p a j -> p j a"), in_=xfs[g][:, u, :])
            # PE transposes: chunk a -> x^T[:, a, tile i]
            px = pxt.tile([P, ND, P], BF16, tag="pxt", name=f"px{i}")
            for a in range(ND):
                nc.tensor.transpose(px[:, a, :], xbt[:, a, :], ident_bf[:])
            nc.vector.tensor_copy(
                out=xT[:, :, i * P:(i + 1) * P], in_=px[:])
            # projection: x^T-stationary, W moving (192 wide)
            ps_p = psmall.tile([P, 3 * H], FP32, tag="small", name=f"psp{i}")
            for a in range(ND):
                nc.tensor.matmul(ps_p[:], xT[:, a, i * P:(i + 1) * P],
                                 w_all[:, a, :],
                                 start=(a == 0), stop=(a == ND - 1))
            # single drain: Q|K|V -> vqk (bf16)
            nc.vector.tensor_copy(out=vqk[:, i, 0:3 * H], in_=ps_p[:])
            # Q,K -> PE transposes -> qkT
            pqk = psmall.tile([H, 2, P], BF16, tag="small", name=f"pqk{i}")
            for u in range(2):
                nc.tensor.transpose(pqk[:, u, :], vqk[:, i, u * H:(u + 1) * H],
                                    ident_bf[:])
            nc.vector.tensor_copy(out=qkT[:, :, i * P:(i + 1) * P], in_=pqk[:])

        # ---- attention ----
        stores = []

        def diag(b, ki):
            return 4 * b <= ki < 4 * b + 4

        class AttnBlock:
            """Attention for one 512-row q-block, emitted in STEPS so the
            exp-gated PV matmuls interleave with later tiles' PE work
            instead of head-of-line blocking the in-order PE queue."""

            def __init__(self, b):
                self.b = b
                self.qlo = 512 * b
                self.pairs = [(2 * j, 2 * j + 1) for j in range(2 * b + 2)]
                self.ps_o = pout.tile([H + 1, 512], FP32, tag="pout",
                                      name=f"pso{b}")
                self.idx = 0
                self.pending = self.s_exp(self.pairs[0])
                self.ob = None

            def s_exp(self, pr):
                b, qlo, ps_o = self.b, self.qlo, self.ps_o
                k0, k1 = pr
                w0 = max(0, k0 * P - qlo)
                w1 = max(0, k1 * P - qlo)
                ps = psS.tile([P, 1024], FP32, tag="psS", name=f"psS{b}_{k0}",
                              uniquify=True)
                pt = ptp.tile([P, 1024], BF16, tag="pt", name=f"pt{b}_{k0}")
                for ki, w, pos in ((k0, w0, 0), (k1, w1, 512)):
                    nc.tensor.matmul(
                        ps[:, pos + w:pos + 512],
                        qkT[:, 1, ki * P:(ki + 1) * P],
                        qkT[:, 0, qlo + w:qlo + 512],
                        start=True, stop=True)
                if b == 0:
                    # fresh PSUM slots: exp only over written regions
                    for ki, w, pos in ((k0, w0, 0), (k1, w1, 512)):
                        nc.scalar.activation(
                            out=pt[:, pos + w:pos + 512],
                            in_=ps[:, pos + w:pos + 512],
                            func=mybir.ActivationFunctionType.Exp,
                            scale=0.125)
                else:
                    # one wide exp; the [512, 512+w1) gap holds stale
                    # (finite) values from an earlier pair and is never
                    # read by PV.
                    nc.scalar.activation(
                        out=pt[:, w0:1024], in_=ps[:, w0:1024],
                        func=mybir.ActivationFunctionType.Exp,
                        scale=0.125)
                for ki, w, pos in ((k0, w0, 0), (k1, w1, 512)):
                    if diag(b, ki):
                        nc.gpsimd.tensor_mul(pt[:, pos + w:pos + w + P],
                                             pt[:, pos + w:pos + w + P],
                                             tri01[:])
                return pt, w0, w1

            def done(self):
                return self.idx >= len(self.pairs)

            def step(self):
                """Emit S/exp of pair idx+1 (pipeline-ahead), then PV of
                pair idx.  On the last step, drain ps_o to bf16 SBUF."""
                b, idx, pairs = self.b, self.idx, self.pairs
                k0, k1 = pairs[idx]
                nxt = (self.s_exp(pairs[idx + 1])
                       if idx + 1 < len(pairs) else None)
                pt, w0, w1 = self.pending
                for ki, w, pos in ((k0, w0, 0), (k1, w1, 512)):
                    nc.tensor.matmul(
                        self.ps_o[:, w:512], vqk[:, ki, 2 * H:3 * H + 1],
                        pt[:, pos + w:pos + 512],
                        start=(idx == 0 and ki == k0),
                        stop=(idx == len(pairs) - 1 and ki == k1))
                self.pending = nxt
                self.idx += 1
                if self.done():
                    self.ob = obp.tile([H + 1, 512], BF16, tag="ob",
                                       name=f"ob{b}")
                    nc.scalar.copy(out=self.ob[:], in_=self.ps_o[:])

        def out_stage(b, ob):
            # Deferred to the tail: these waits must never head-of-line
            # block streaming work.
            pot = psmall.tile([P, 4, VA], BF16, tag="small", name=f"pot{b}")
            for j in range(4):
                nc.tensor.transpose(pot[:, j, 0:H + 1],
                                    ob[:, j * P:(j + 1) * P],
                                    ident_bf[0:H + 1, 0:H + 1])
            # Only TWO DVE queue entries here (copy + recip) -- they park in
            # the 4-deep wait queue without stalling DVE.SEQ for the
            # streaming casts behind them; the rescales run on idle gpsimd
            # from SBUF.
            ot = osbp.tile([P, 4, H + 1], BF16, tag="ot", name=f"ot{b}")
            nc.vector.tensor_copy(out=ot[:], in_=pot[:, :, 0:H + 1])
            rec = recp.tile([P, 4], FP32, tag="rec", name=f"rec{b}")
            nc.vector.reciprocal(rec[:], ot[:, :, H])
            osb = osbp.tile([P, 4, H], BF16, tag="osb", name=f"osb{b}")
            for j in range(4):
                nc.gpsimd.tensor_scalar_mul(osb[:, j, :], ot[:, j, 0:H],
                                            rec[:, j:j + 1])
            stores.append(
                (out_d.rearrange("(b j p) h -> b p j h", p=P, j=4)[b], osb))

        # Emission tracks data arrival: a block starts ~2 tiles after its
        # last q-tile's qkT; pair-steps interleave one-per-tile-slot so
        # the exp-gated PVs never clump ahead of later tiles' PE work;
        # out stages slot in after their block finishes.
        START_AT = {5: 0, 8: 1, 12: 2}
        active = []
        finished = []
        staged = 0

        def run_steps(budget):
            nonlocal staged
            n = 0
            while active and n < budget:
                blk = active[0]
                blk.step()
                n += 1
                if blk.done():
                    finished.append(active.pop(0))
            # emit a finished block's out stage once its successor started
            if finished and len(finished) + len(active) > staged + 1:
                blk = finished[staged]
                if blk is finished[staged]:
                    out_stage(blk.b, blk.ob)
                    staged += 1

        for i in range(NT):
            tile_work(i)
            if i in START_AT:
                active.append(AttnBlock(START_AT[i]))
            run_steps(1 if i < 12 else 2)
        active.append(AttnBlock(3))
        while active:
            run_steps(100)
        for blk in finished[staged:]:
            out_stage(blk.b, blk.ob)

        for dst, osb in stores:
            nc.sync.dma_start(out=dst, in_=osb[:])


def _run(inputs, trace=False, **kw):
    global _compiled
    if _compiled is None:
        _compiled = _build()
    nc = _compiled
    x = np.ascontiguousarray(inputs["x"], dtype=np.float32)
    wq = np.ascontiguousarray(inputs["Wq"], dtype=np.float32)
    wk = np.ascontiguousarray(inputs["Wk"], dtype=np.float32)
    wv = np.ascontiguousarray(inputs["Wv"], dtype=np.float32)
    in_maps = [
        {"x": np.ascontiguousarray(x[i]), "Wq": wq, "Wk": wk, "Wv": wv}
        for i in range(B)
    ]
    res = run_bass_kernel_spmd(nc, in_maps, core_ids=list(range(B)),
                               trace=trace, **kw)
    out = np.stack(
        [np.asarray(res.results[i]["out"]).astype(np.float32) for i in range(B)],
        axis=0)
    return out, res


def kernel(x, Wq, Wk, Wv):
    out, _ = _run({"x": x, "Wq": Wq, "Wk": Wk, "Wv": Wv})
    return out


# revision 37
# speedup vs baseline: 1.0884x; 1.0600x over previous
"""Single-head causal attention on 8 TRN2 NeuronCores.

Problem: x [8, 2048, 1024] f32, Wq/Wk/Wv [1024, 64] f32.
  q = x @ Wq ; k = x @ Wk ; v = x @ Wv        (per batch)
  out = softmax(causal(q k^T / 8)) @ v        [8, 2048, 64]

Sharding: data-parallel over batch -- core i handles batch element i.
No collectives needed.

Per-core kernel (bf16 compute, f32 accumulate), 128-token-tile pipeline:
  1. W loads use the natural row-contiguous layout (2KB descriptors, no
     sub-512B DMA penalty); the d-contraction is chunked INTERLEAVED
     (chunk a = {d : d = 8p + a}) so the natural layout needs no
     on-chip weight transpose -- gpsimd packs [Wq|Wk|Wv] to bf16.
  2. x streams per 256-token group; each 128-tile is cast f32->bf16 on
     DVE directly into the interleaved layout, transposed on the PE
     (8x [128,128] identity matmuls -> PSUM bf16), and copied to the
     x^T SBUF pool by DVE.
  3. Projections are x-stationary: lhsT = x^T tile-chunk, moving
     rhs = [Wq|Wk|Wv] (192 wide) -> PSUM [t,192] in 8 matmuls/tile
     (1536 PE cycles vs 2048 for the W-stationary form).
  4. Q,K land t-major; one DMA-xbar transpose per tile ([t,128] ->
     [qk,t], 8 ucode tiles = ~112ns of DMA) yields Q^T/K^T rows with
     h on partitions. V stays t-major (what PV wants) and is copied
     into V_aug with a ones column (softmax denominator for free).
  5. Attention per 512-row q-block: S^T[tk,tq] = K^T_tile.T @ Q^T
     (contraction h); exp on ACT in k-tile PAIRS (halves the ~185ns
     per-instruction access-latency overhead); causal diagonal via a
     multiplicative 0/1 bf16 mask on DVE; PV accumulates
     out^T[65,tq] += V_aug.T @ P^T in PSUM, row 64 = denominators.
  6. Output: PSUM -> bf16 SBUF copy (gpsimd), PE-transpose back to
     [tq,65], reciprocal-rescale (DVE recip + gpsimd scale), bf16
     store (f32 upcast happens host-side after gather).

Engine budget: PE ~76k cycles (transposes 16.4k, proj 24.6k, S 16.9k,
PV 16.9k, out 1k) is the critical resource; DMA ~29us (x 23.3 =
roofline, W 2.2, qk-xbar 1.8, stores 1.5); ACT owns exp (~20us); DVE
casts/copies/masks (~22us); gpsimd does the PSUM drains (~11us).
"""

import numpy as np

import concourse.bass as bass
import concourse.tile as tile
from concourse import bacc, mybir
from concourse.bass_utils import run_bass_kernel_spmd

B, T, D, H = 8, 2048, 1024, 64
P = 128            # partitions / tile edge
ND = D // P        # 8 d-chunks (interleaved: chunk a = {d : d = 8p + a})
NT = T // P        # 16 token tiles
NB = T // 512      # 4 q-blocks of 512 rows
VA = 80            # v_aug padded k-tile stride

FP32 = mybir.dt.float32
BF16 = mybir.dt.bfloat16

_compiled = None


def _build():
    nc = bacc.Bacc("TRN2", target_bir_lowering=False, debug=False, num_devices=8)

    x_d = nc.dram_tensor("x", [T, D], FP32, kind="ExternalInput").ap()
    wq_d = nc.dram_tensor("Wq", [D, H], FP32, kind="ExternalInput").ap()
    wk_d = nc.dram_tensor("Wk", [D, H], FP32, kind="ExternalInput").ap()
    wv_d = nc.dram_tensor("Wv", [D, H], FP32, kind="ExternalInput").ap()
    out_d = nc.dram_tensor("out", [T, H], BF16, kind="ExternalOutput").ap()

    with tile.TileContext(nc) as tc:
        _kernel(tc, out_d, x_d, wq_d, wk_d, wv_d)

    nc.compile()
    return nc


def _kernel(tc, out_d, x_d, wq_d, wk_d, wv_d):
    nc = tc.nc
    from contextlib import ExitStack

    ctx = ExitStack()
    with ctx:
        const = ctx.enter_context(tc.tile_pool(name="const", bufs=1))
        wstage = ctx.enter_context(tc.tile_pool(name="wstage", bufs=3))
        xload = ctx.enter_context(tc.tile_pool(name="xload", bufs=6))
        xbtp = ctx.enter_context(tc.tile_pool(name="xbtp", bufs=4))
        xtp = ctx.enter_context(tc.tile_pool(name="xtp", bufs=1))
        qkp = ctx.enter_context(tc.tile_pool(name="qkp", bufs=1))
        qksp = ctx.enter_context(tc.tile_pool(name="qksp", bufs=2))
        vsb = ctx.enter_context(tc.tile_pool(name="vsb", bufs=1))
        ptp = ctx.enter_context(tc.tile_pool(name="ptp", bufs=3))
        obp = ctx.enter_context(tc.tile_pool(name="obp", bufs=2))
        osbp = ctx.enter_context(tc.tile_pool(name="osbp", bufs=2))
        recp = ctx.enter_context(tc.tile_pool(name="recp", bufs=2))
        pxt = ctx.enter_context(tc.tile_pool(name="pxt", bufs=1, space="PSUM"))
        psS = ctx.enter_context(tc.tile_pool(name="psS", bufs=2, space="PSUM"))
        pout = ctx.enter_context(tc.tile_pool(name="pout", bufs=1, space="PSUM"))
        psmall = ctx.enter_context(tc.tile_pool(name="psmall", bufs=2, space="PSUM"))

        # ---- loads: first x group, then weights, then remaining x ----
        # (x group 0 first so the cast/transpose pipeline starts ~2us
        # earlier; W only gates the projections, which queue behind.)
        x_r = x_d.rearrange("(g u p) d -> g p u d", p=P, u=2)
        xfs = {}

        def load_x(g):
            xf = xload.tile([P, 2, D], FP32, tag="xf", name=f"xf{g}")
            nc.sync.dma_start(out=xf[:], in_=x_r[g])
            xfs[g] = xf

        load_x(0)

        # Weight loads use the natural row-contiguous layout (2KB
        # descriptors; no sub-512B DMA penalty); chunk a of the interleaved
        # contraction is the partition-slice [:, a, :], so no weight
        # transpose is ever needed.
        w_all = const.tile([P, ND, 3 * H], BF16)   # [Wq | Wk | Wv] per slot
        wnats = []
        for w_dram, name in ((wq_d, "wq"), (wk_d, "wk"), (wv_d, "wv")):
            wn = wstage.tile([P, ND, H], FP32, tag="wstage", name=f"stg_{name}")
            nc.sync.dma_start(out=wn[:], in_=w_dram.rearrange(
                "(p a) h -> p a h", p=P))
            wnats.append(wn)
        for g in range(1, NT // 2):
            load_x(g)

        # Weight pack on ACT: its queue is empty until the first exps
        # (~12us), so waiting on the W loads here blocks nothing.
        for j, wn in enumerate(wnats):
            nc.scalar.copy(out=w_all[:, :, j * H:(j + 1) * H], in_=wn[:])

        # ---- constants ----
        ident_bf = const.tile([P, P], BF16)
        from concourse.masks import make_identity
        make_identity(nc, ident_bf[:])

        # 0/1 upper-triangular (incl. diagonal) bf16 mask in [tk, tq]
        # orientation: valid when tq >= tk (col >= row).
        tri01 = const.tile([P, P], BF16)
        nc.gpsimd.memset(tri01[:], 1.0)
        nc.gpsimd.affine_select(
            out=tri01[:], in_=tri01[:],
            compare_op=mybir.AluOpType.is_ge,
            fill=0.0, base=0,
            pattern=[[1, P]], channel_multiplier=-1)

        # Per-tile projection results, persistent: [:, i, 0:64] = Q,
        # [64:128] = K (both t-major, transposed later on the PE),
        # [128:192] = V, col 192 = 1.0 (softmax-denominator row for the
        # PV matmul's 65-row lhsT).  One DVE copy drains all three.
        vqk = vsb.tile([P, NT, 200], BF16)
        nc.gpsimd.memset(vqk[:, :, 3 * H:3 * H + 1], 1.0)


        # persistent SBUF state
        xT = xtp.tile([P, ND, T], BF16)      # x^T, interleaved chunks
        # Q^T/K^T ([:, 0] = Q^T, [:, 1] = K^T), h on partitions 0:64 --
        # produced by per-tile PE transposes (the DMA-xbar alternative
        # serializes against the x stream on the shared DMA engines and
        # poisons the in-order HWDGE queues with its waits).
        qkT = qkp.tile([H, 2, T], BF16)

        # ---- per-tile pipeline, split front/back and emitted with a
        # one-tile skew (front(i+1) before back(i)) so the PE round-trips
        # inside a tile (transpose->copy->proj->drain->qk-transpose) never
        # expose their latency on the in-order DVE queue.
        def tile_front(i):
            g, u = divmod(i, 2)
            # cast f32 -> bf16 into the interleaved (a, j) layout:
            # element d of the tile lands at [a = d % 8, j = d // 8].
            # Every third tile casts on the (otherwise idle) gpsimd so the
            # DVE keeps up with its PSUM-drain copies.
            xbt = xbtp.tile([P, ND, P], BF16, tag="xbt", name=f"xbt{i}")
            cast_eng = nc.gpsimd if i % 3 == 2 else nc.vector
            cast_eng.tensor_copy(
                out=xbt[:].rearrange("p a j -> p j a"), in_=xfs[g][:, u, :])
            # PE transposes: chunk a -> x^T[:, a, tile i]
            px = pxt.tile([P, ND, P], BF16, tag="pxt", name=f"px{i}")
            for a in range(ND):
                nc.tensor.transpose(px[:, a, :], xbt[:, a, :], ident_bf[:])
            nc.vector.tensor_copy(
                out=xT[:, :, i * P:(i + 1) * P], in_=px[:])

        def tile_back(i):
            # projection: x^T-stationary, W moving (192 wide)
            ps_p = psmall.tile([P, 3 * H], FP32, tag="small", name=f"psp{i}")
            for a in range(ND):
                nc.tensor.matmul(ps_p[:], xT[:, a, i * P:(i + 1) * P],
                                 w_all[:, a, :],
                                 start=(a == 0), stop=(a == ND - 1))
            # single drain: Q|K|V -> vqk (bf16)
            nc.vector.tensor_copy(out=vqk[:, i, 0:3 * H], in_=ps_p[:])
            # Q,K -> PE transposes -> qkT
            pqk = psmall.tile([H, 2, P], BF16, tag="small", name=f"pqk{i}")
            for u in range(2):
                nc.tensor.transpose(pqk[:, u, :], vqk[:, i, u * H:(u + 1) * H],
                                    ident_bf[:])
            nc.vector.tensor_copy(out=qkT[:, :, i * P:(i + 1) * P], in_=pqk[:])

        # ---- attention ----
        stores = []

        def diag(b, ki):
            return 4 * b <= ki < 4 * b + 4

        class AttnBlock:
            """Attention for one 512-row q-block, emitted in STEPS so the
            exp-gated PV matmuls interleave with later tiles' PE work
            instead of head-of-line blocking the in-order PE queue."""

            def __init__(self, b):
                self.b = b
                self.qlo = 512 * b
                self.pairs = [(2 * j, 2 * j + 1) for j in range(2 * b + 2)]
                self.ps_o = pout.tile([H + 1, 512], FP32, tag="pout",
                                      name=f"pso{b}")
                self.idx = 0
                self.pending = self.s_exp(self.pairs[0])
                self.ob = None

            def s_exp(self, pr):
                b, qlo, ps_o = self.b, self.qlo, self.ps_o
                k0, k1 = pr
                w0 = max(0, k0 * P - qlo)
                w1 = max(0, k1 * P - qlo)
                ps = psS.tile([P, 1024], FP32, tag="psS", name=f"psS{b}_{k0}",
                              uniquify=True)
                pt = ptp.tile([P, 1024], BF16, tag="pt", name=f"pt{b}_{k0}")
                for ki, w, pos in ((k0, w0, 0), (k1, w1, 512)):
                    nc.tensor.matmul(
                        ps[:, pos + w:pos + 512],
                        qkT[:, 1, ki * P:(ki + 1) * P],
                        qkT[:, 0, qlo + w:qlo + 512],
                        start=True, stop=True)
                if b == 0:
                    # fresh PSUM slots: exp only over written regions
                    for ki, w, pos in ((k0, w0, 0), (k1, w1, 512)):
                        nc.scalar.activation(
                            out=pt[:, pos + w:pos + 512],
                            in_=ps[:, pos + w:pos + 512],
                            func=mybir.ActivationFunctionType.Exp,
                            scale=0.125)
                else:
                    # one wide exp; the [512, 512+w1) gap holds stale
                    # (finite) values from an earlier pair and is never
                    # read by PV.
                    nc.scalar.activation(
                        out=pt[:, w0:1024], in_=ps[:, w0:1024],
                        func=mybir.ActivationFunctionType.Exp,
                        scale=0.125)
                for ki, w, pos in ((k0, w0, 0), (k1, w1, 512)):
                    if diag(b, ki):
                        nc.gpsimd.tensor_mul(pt[:, pos + w:pos + w + P],
                                             pt[:, pos + w:pos + w + P],
                                             tri01[:])
                return pt, w0, w1

            def done(self):
                return self.idx >= len(self.pairs)

            def step(self):
                """Emit S/exp of pair idx+1 (pipeline-ahead), then PV of
                pair idx.  On the last step, drain ps_o to bf16 SBUF."""
                b, idx, pairs = self.b, self.idx, self.pairs
                k0, k1 = pairs[idx]
                nxt = (self.s_exp(pairs[idx + 1])
                       if idx + 1 < len(pairs) else None)
                pt, w0, w1 = self.pending
                for ki, w, pos in ((k0, w0, 0), (k1, w1, 512)):
                    nc.tensor.matmul(
                        self.ps_o[:, w:512], vqk[:, ki, 2 * H:3 * H + 1],
                        pt[:, pos + w:pos + 512],
                        start=(idx == 0 and ki == k0),
                        stop=(idx == len(pairs) - 1 and ki == k1))
                self.pending = nxt
                self.idx += 1
                if self.done():
                    self.ob = obp.tile([H + 1, 512], BF16, tag="ob",
                                       name=f"ob{b}")
                    nc.scalar.copy(out=self.ob[:], in_=self.ps_o[:])

        def out_stage(b, ob):
            # Deferred to the tail: these waits must never head-of-line
            # block streaming work.
            pot = psmall.tile([P, 4, VA], BF16, tag="small", name=f"pot{b}")
            for j in range(4):
                nc.tensor.transpose(pot[:, j, 0:H + 1],
                                    ob[:, j * P:(j + 1) * P],
                                    ident_bf[0:H + 1, 0:H + 1])
            # Only TWO DVE queue entries here (copy + recip) -- they park in
            # the 4-deep wait queue without stalling DVE.SEQ for the
            # streaming casts behind them; the rescales run on idle gpsimd
            # from SBUF.
            ot = osbp.tile([P, 4, H + 1], BF16, tag="ot", name=f"ot{b}")
            nc.vector.tensor_copy(out=ot[:], in_=pot[:, :, 0:H + 1])
            rec = recp.tile([P, 4], FP32, tag="rec", name=f"rec{b}")
            nc.vector.reciprocal(rec[:], ot[:, :, H])
            osb = osbp.tile([P, 4, H], BF16, tag="osb", name=f"osb{b}")
            for j in range(4):
                nc.gpsimd.tensor_scalar_mul(osb[:, j, :], ot[:, j, 0:H],
                                            rec[:, j:j + 1])
            stores.append(
                (out_d.rearrange("(b j p) h -> b p j h", p=P, j=4)[b], osb))

        # Emission tracks data arrival: a block starts ~2 tiles after its
        # last q-tile's qkT; pair-steps interleave one-per-tile-slot so
        # the exp-gated PVs never clump ahead of later tiles' PE work;
        # out stages slot in after their block finishes.
        START_AT = {5: 0, 8: 1, 12: 2}
        active = []
        finished = []
        staged = 0

        def run_steps(budget):
            nonlocal staged
            n = 0
            while active and n < budget:
                blk = active[0]
                blk.step()
                n += 1
                if blk.done():
                    finished.append(active.pop(0))
            # emit a finished block's out stage once its successor started
            if finished and len(finished) + len(active) > staged + 1:
                blk = finished[staged]
                if blk is finished[staged]:
                    out_stage(blk.b, blk.ob)
                    staged += 1

        for i in range(NT):
            tile_front(i)
            if i >= 1:
                tile_back(i - 1)
            if i in START_AT:
                active.append(AttnBlock(START_AT[i]))
            run_steps(1 if i < 12 else 2)
        tile_back(NT - 1)
        active.append(AttnBlock(3))
        while active:
            run_steps(100)
        for blk in finished[staged:]:
            out_stage(blk.b, blk.ob)

        for dst, osb in stores:
            nc.sync.dma_start(out=dst, in_=osb[:])


def _run(inputs, trace=False, **kw):
    global _compiled
    if _compiled is None:
        _compiled = _build()
    nc = _compiled
    x = np.ascontiguousarray(inputs["x"], dtype=np.float32)
    wq = np.ascontiguousarray(inputs["Wq"], dtype=np.float32)
    wk = np.ascontiguousarray(inputs["Wk"], dtype=np.float32)
    wv = np.ascontiguousarray(inputs["Wv"], dtype=np.float32)
    in_maps = [
        {"x": np.ascontiguousarray(x[i]), "Wq": wq, "Wk": wk, "Wv": wv}
        for i in range(B)
    ]
    res = run_bass_kernel_spmd(nc, in_maps, core_ids=list(range(B)),
                               trace=trace, **kw)
    out = np.stack(
        [np.asarray(res.results[i]["out"]).astype(np.float32) for i in range(B)],
        axis=0)
    return out, res


def kernel(x, Wq, Wk, Wv):
    out, _ = _run({"x": x, "Wq": Wq, "Wk": Wk, "Wv": Wv})
    return out


# revision 38
# speedup vs baseline: 1.0890x; 1.0005x over previous
"""Single-head causal attention on 8 TRN2 NeuronCores.

Problem: x [8, 2048, 1024] f32, Wq/Wk/Wv [1024, 64] f32.
  q = x @ Wq ; k = x @ Wk ; v = x @ Wv        (per batch)
  out = softmax(causal(q k^T / 8)) @ v        [8, 2048, 64]

Sharding: data-parallel over batch -- core i handles batch element i.
No collectives needed.

Per-core kernel (bf16 compute, f32 accumulate), 128-token-tile pipeline:
  1. W loads use the natural row-contiguous layout (2KB descriptors, no
     sub-512B DMA penalty); the d-contraction is chunked INTERLEAVED
     (chunk a = {d : d = 8p + a}) so the natural layout needs no
     on-chip weight transpose -- gpsimd packs [Wq|Wk|Wv] to bf16.
  2. x streams per 256-token group; each 128-tile is cast f32->bf16 on
     DVE directly into the interleaved layout, transposed on the PE
     (8x [128,128] identity matmuls -> PSUM bf16), and copied to the
     x^T SBUF pool by DVE.
  3. Projections are x-stationary: lhsT = x^T tile-chunk, moving
     rhs = [Wq|Wk|Wv] (192 wide) -> PSUM [t,192] in 8 matmuls/tile
     (1536 PE cycles vs 2048 for the W-stationary form).
  4. Q,K land t-major; one DMA-xbar transpose per tile ([t,128] ->
     [qk,t], 8 ucode tiles = ~112ns of DMA) yields Q^T/K^T rows with
     h on partitions. V stays t-major (what PV wants) and is copied
     into V_aug with a ones column (softmax denominator for free).
  5. Attention per 512-row q-block: S^T[tk,tq] = K^T_tile.T @ Q^T
     (contraction h); exp on ACT in k-tile PAIRS (halves the ~185ns
     per-instruction access-latency overhead); causal diagonal via a
     multiplicative 0/1 bf16 mask on DVE; PV accumulates
     out^T[65,tq] += V_aug.T @ P^T in PSUM, row 64 = denominators.
  6. Output: PSUM -> bf16 SBUF copy (gpsimd), PE-transpose back to
     [tq,65], reciprocal-rescale (DVE recip + gpsimd scale), bf16
     store (f32 upcast happens host-side after gather).

Engine budget: PE ~76k cycles (transposes 16.4k, proj 24.6k, S 16.9k,
PV 16.9k, out 1k) is the critical resource; DMA ~29us (x 23.3 =
roofline, W 2.2, qk-xbar 1.8, stores 1.5); ACT owns exp (~20us); DVE
casts/copies/masks (~22us); gpsimd does the PSUM drains (~11us).
"""

import numpy as np

import concourse.bass as bass
import concourse.tile as tile
from concourse import bacc, mybir
from concourse.bass_utils import run_bass_kernel_spmd

B, T, D, H = 8, 2048, 1024, 64
P = 128            # partitions / tile edge
ND = D // P        # 8 d-chunks (interleaved: chunk a = {d : d = 8p + a})
NT = T // P        # 16 token tiles
NB = T // 512      # 4 q-blocks of 512 rows
VA = 80            # v_aug padded k-tile stride

FP32 = mybir.dt.float32
BF16 = mybir.dt.bfloat16

_compiled = None


def _build():
    nc = bacc.Bacc("TRN2", target_bir_lowering=False, debug=False, num_devices=8)

    x_d = nc.dram_tensor("x", [T, D], FP32, kind="ExternalInput").ap()
    wq_d = nc.dram_tensor("Wq", [D, H], FP32, kind="ExternalInput").ap()
    wk_d = nc.dram_tensor("Wk", [D, H], FP32, kind="ExternalInput").ap()
    wv_d = nc.dram_tensor("Wv", [D, H], FP32, kind="ExternalInput").ap()
    out_d = nc.dram_tensor("out", [T, H], BF16, kind="ExternalOutput").ap()

    with tile.TileContext(nc) as tc:
        _kernel(tc, out_d, x_d, wq_d, wk_d, wv_d)

    nc.compile()
    return nc


def _kernel(tc, out_d, x_d, wq_d, wk_d, wv_d):
    nc = tc.nc
    from contextlib import ExitStack

    ctx = ExitStack()
    with ctx:
        const = ctx.enter_context(tc.tile_pool(name="const", bufs=1))
        wstage = ctx.enter_context(tc.tile_pool(name="wstage", bufs=3))
        xload = ctx.enter_context(tc.tile_pool(name="xload", bufs=6))
        xbtp = ctx.enter_context(tc.tile_pool(name="xbtp", bufs=4))
        xtp = ctx.enter_context(tc.tile_pool(name="xtp", bufs=1))
        qkp = ctx.enter_context(tc.tile_pool(name="qkp", bufs=1))
        qksp = ctx.enter_context(tc.tile_pool(name="qksp", bufs=2))
        vsb = ctx.enter_context(tc.tile_pool(name="vsb", bufs=1))
        ptp = ctx.enter_context(tc.tile_pool(name="ptp", bufs=3))
        obp = ctx.enter_context(tc.tile_pool(name="obp", bufs=2))
        osbp = ctx.enter_context(tc.tile_pool(name="osbp", bufs=2))
        recp = ctx.enter_context(tc.tile_pool(name="recp", bufs=2))
        pxt = ctx.enter_context(tc.tile_pool(name="pxt", bufs=1, space="PSUM"))
        psS = ctx.enter_context(tc.tile_pool(name="psS", bufs=2, space="PSUM"))
        pout = ctx.enter_context(tc.tile_pool(name="pout", bufs=1, space="PSUM"))
        psmall = ctx.enter_context(tc.tile_pool(name="psmall", bufs=2, space="PSUM"))

        # ---- loads: first x group, then weights, then remaining x ----
        # (x group 0 first so the cast/transpose pipeline starts ~2us
        # earlier; W only gates the projections, which queue behind.)
        x_r = x_d.rearrange("(g u p) d -> g p u d", p=P, u=2)
        xfs = {}

        def load_x(g):
            xf = xload.tile([P, 2, D], FP32, tag="xf", name=f"xf{g}")
            nc.sync.dma_start(out=xf[:], in_=x_r[g])
            xfs[g] = xf

        load_x(0)

        # Weight loads use the natural row-contiguous layout (2KB
        # descriptors; no sub-512B DMA penalty); chunk a of the interleaved
        # contraction is the partition-slice [:, a, :], so no weight
        # transpose is ever needed.
        w_all = const.tile([P, ND, 3 * H], BF16)   # [Wq | Wk | Wv] per slot
        wnats = []
        for w_dram, name in ((wq_d, "wq"), (wk_d, "wk"), (wv_d, "wv")):
            wn = wstage.tile([P, ND, H], FP32, tag="wstage", name=f"stg_{name}")
            nc.sync.dma_start(out=wn[:], in_=w_dram.rearrange(
                "(p a) h -> p a h", p=P))
            wnats.append(wn)
        for g in range(1, NT // 2):
            load_x(g)

        # Weight pack on ACT: its queue is empty until the first exps
        # (~12us), so waiting on the W loads here blocks nothing.
        for j, wn in enumerate(wnats):
            nc.scalar.copy(out=w_all[:, :, j * H:(j + 1) * H], in_=wn[:])

        # ---- constants ----
        ident_bf = const.tile([P, P], BF16)
        from concourse.masks import make_identity
        make_identity(nc, ident_bf[:])

        # 0/1 upper-triangular (incl. diagonal) bf16 mask in [tk, tq]
        # orientation: valid when tq >= tk (col >= row).
        tri01 = const.tile([P, P], BF16)
        nc.gpsimd.memset(tri01[:], 1.0)
        nc.gpsimd.affine_select(
            out=tri01[:], in_=tri01[:],
            compare_op=mybir.AluOpType.is_ge,
            fill=0.0, base=0,
            pattern=[[1, P]], channel_multiplier=-1)

        # Per-tile projection results, persistent: [:, i, 0:64] = Q,
        # [64:128] = K (both t-major, transposed later on the PE),
        # [128:192] = V, col 192 = 1.0 (softmax-denominator row for the
        # PV matmul's 65-row lhsT).  One DVE copy drains all three.
        vqk = vsb.tile([P, NT, 200], BF16)
        nc.gpsimd.memset(vqk[:, :, 3 * H:3 * H + 1], 1.0)


        # persistent SBUF state
        xT = xtp.tile([P, ND, T], BF16)      # x^T, interleaved chunks
        # Q^T/K^T ([:, 0] = Q^T, [:, 1] = K^T), h on partitions 0:64 --
        # produced by per-tile PE transposes (the DMA-xbar alternative
        # serializes against the x stream on the shared DMA engines and
        # poisons the in-order HWDGE queues with its waits).
        qkT = qkp.tile([H, 2, T], BF16)

        # ---- per-tile pipeline, split front/back and emitted with a
        # one-tile skew (front(i+1) before back(i)) so the PE round-trips
        # inside a tile (transpose->copy->proj->drain->qk-transpose) never
        # expose their latency on the in-order DVE queue.
        def tile_front(i):
            g, u = divmod(i, 2)
            # cast f32 -> bf16 into the interleaved (a, j) layout:
            # element d of the tile lands at [a = d % 8, j = d // 8].
            # Every third tile casts on the (otherwise idle) gpsimd so the
            # DVE keeps up with its PSUM-drain copies.
            xbt = xbtp.tile([P, ND, P], BF16, tag="xbt", name=f"xbt{i}")
            cast_eng = nc.gpsimd if i % 3 == 2 else nc.vector
            cast_eng.tensor_copy(
                out=xbt[:].rearrange("p a j -> p j a"), in_=xfs[g][:, u, :])
            # PE transposes: chunk a -> x^T[:, a, tile i]
            px = pxt.tile([P, ND, P], BF16, tag="pxt", name=f"px{i}")
            for a in range(ND):
                nc.tensor.transpose(px[:, a, :], xbt[:, a, :], ident_bf[:])
            nc.vector.tensor_copy(
                out=xT[:, :, i * P:(i + 1) * P], in_=px[:])

        def tile_back(i):
            # projection: x^T-stationary, W moving (192 wide)
            ps_p = psmall.tile([P, 3 * H], FP32, tag="small", name=f"psp{i}")
            for a in range(ND):
                nc.tensor.matmul(ps_p[:], xT[:, a, i * P:(i + 1) * P],
                                 w_all[:, a, :],
                                 start=(a == 0), stop=(a == ND - 1))
            # single drain: Q|K|V -> vqk (bf16)
            nc.vector.tensor_copy(out=vqk[:, i, 0:3 * H], in_=ps_p[:])
            # Q,K -> PE transposes -> qkT
            pqk = psmall.tile([H, 2, P], BF16, tag="small", name=f"pqk{i}")
            for u in range(2):
                nc.tensor.transpose(pqk[:, u, :], vqk[:, i, u * H:(u + 1) * H],
                                    ident_bf[:])
            nc.vector.tensor_copy(out=qkT[:, :, i * P:(i + 1) * P], in_=pqk[:])

        # ---- attention ----
        stores = []

        def diag(b, ki):
            return 4 * b <= ki < 4 * b + 4

        class AttnBlock:
            """Attention for one 512-row q-block, emitted in STEPS so the
            exp-gated PV matmuls interleave with later tiles' PE work
            instead of head-of-line blocking the in-order PE queue."""

            def __init__(self, b):
                self.b = b
                self.qlo = 512 * b
                self.pairs = [(2 * j, 2 * j + 1) for j in range(2 * b + 2)]
                self.ps_o = pout.tile([H + 1, 512], FP32, tag="pout",
                                      name=f"pso{b}")
                self.idx = 0
                self.pending = self.s_exp(self.pairs[0])
                self.ob = None

            def s_exp(self, pr):
                b, qlo, ps_o = self.b, self.qlo, self.ps_o
                k0, k1 = pr
                w0 = max(0, k0 * P - qlo)
                w1 = max(0, k1 * P - qlo)
                ps = psS.tile([P, 1024], FP32, tag="psS", name=f"psS{b}_{k0}",
                              uniquify=True)
                pt = ptp.tile([P, 1024], BF16, tag="pt", name=f"pt{b}_{k0}")
                for ki, w, pos in ((k0, w0, 0), (k1, w1, 512)):
                    nc.tensor.matmul(
                        ps[:, pos + w:pos + 512],
                        qkT[:, 1, ki * P:(ki + 1) * P],
                        qkT[:, 0, qlo + w:qlo + 512],
                        start=True, stop=True)
                if b == 0:
                    # fresh PSUM slots: exp only over written regions
                    for ki, w, pos in ((k0, w0, 0), (k1, w1, 512)):
                        nc.scalar.activation(
                            out=pt[:, pos + w:pos + 512],
                            in_=ps[:, pos + w:pos + 512],
                            func=mybir.ActivationFunctionType.Exp,
                            scale=0.125)
                else:
                    # one wide exp; the [512, 512+w1) gap holds stale
                    # (finite) values from an earlier pair and is never
                    # read by PV.
                    nc.scalar.activation(
                        out=pt[:, w0:1024], in_=ps[:, w0:1024],
                        func=mybir.ActivationFunctionType.Exp,
                        scale=0.125)
                for ki, w, pos in ((k0, w0, 0), (k1, w1, 512)):
                    if diag(b, ki):
                        nc.gpsimd.tensor_mul(pt[:, pos + w:pos + w + P],
                                             pt[:, pos + w:pos + w + P],
                                             tri01[:])
                return pt, w0, w1

            def done(self):
                return self.idx >= len(self.pairs)

            def step(self):
                """Emit S/exp of pair idx+1 (pipeline-ahead), then PV of
                pair idx.  On the last step, drain ps_o to bf16 SBUF."""
                b, idx, pairs = self.b, self.idx, self.pairs
                k0, k1 = pairs[idx]
                nxt = (self.s_exp(pairs[idx + 1])
                       if idx + 1 < len(pairs) else None)
                pt, w0, w1 = self.pending
                for ki, w, pos in ((k0, w0, 0), (k1, w1, 512)):
                    nc.tensor.matmul(
                        self.ps_o[:, w:512], vqk[:, ki, 2 * H:3 * H + 1],
                        pt[:, pos + w:pos + 512],
                        start=(idx == 0 and ki == k0),
                        stop=(idx == len(pairs) - 1 and ki == k1))
                self.pending = nxt
                self.idx += 1
                if self.done():
                    self.ob = obp.tile([H + 1, 512], BF16, tag="ob",
                                       name=f"ob{b}")
                    nc.scalar.copy(out=self.ob[:], in_=self.ps_o[:])

        def out_stage(b, ob):
            # Deferred to the tail: these waits must never head-of-line
            # block streaming work.
            pot = psmall.tile([P, 4, VA], BF16, tag="small", name=f"pot{b}")
            for j in range(4):
                nc.tensor.transpose(pot[:, j, 0:H + 1],
                                    ob[:, j * P:(j + 1) * P],
                                    ident_bf[0:H + 1, 0:H + 1])
            # Only TWO DVE queue entries here (copy + recip) -- they park in
            # the 4-deep wait queue without stalling DVE.SEQ for the
            # streaming casts behind them; the rescales run on idle gpsimd
            # from SBUF.
            ot = osbp.tile([P, 4, H + 1], BF16, tag="ot", name=f"ot{b}")
            nc.vector.tensor_copy(out=ot[:], in_=pot[:, :, 0:H + 1])
            rec = recp.tile([P, 4], FP32, tag="rec", name=f"rec{b}")
            nc.vector.reciprocal(rec[:], ot[:, :, H])
            osb = osbp.tile([P, 4, H], BF16, tag="osb", name=f"osb{b}")
            for j in range(4):
                nc.gpsimd.tensor_scalar_mul(osb[:, j, :], ot[:, j, 0:H],
                                            rec[:, j:j + 1])
            stores.append(
                (out_d.rearrange("(b j p) h -> b p j h", p=P, j=4)[b], osb))

        # Emission tracks data arrival: a block starts ~2 tiles after its
        # last q-tile's qkT; pair-steps interleave one-per-tile-slot so
        # the exp-gated PVs never clump ahead of later tiles' PE work;
        # out stages slot in after their block finishes.
        START_AT = {6: 0, 9: 1, 13: 2}
        active = []
        finished = []
        staged = 0

        def run_steps(budget):
            nonlocal staged
            n = 0
            while active and n < budget:
                blk = active[0]
                blk.step()
                n += 1
                if blk.done():
                    finished.append(active.pop(0))
            # emit a finished block's out stage once its successor started
            if finished and len(finished) + len(active) > staged + 1:
                blk = finished[staged]
                if blk is finished[staged]:
                    out_stage(blk.b, blk.ob)
                    staged += 1

        SKEW = 2
        for i in range(NT):
            tile_front(i)
            if i >= SKEW:
                tile_back(i - SKEW)
            if i in START_AT:
                active.append(AttnBlock(START_AT[i]))
            run_steps(1 if i < 12 else 2)
        for i in range(NT - SKEW, NT):
            tile_back(i)
        active.append(AttnBlock(3))
        while active:
            run_steps(100)
        for blk in finished[staged:]:
            out_stage(blk.b, blk.ob)

        for dst, osb in stores:
            nc.sync.dma_start(out=dst, in_=osb[:])


def _run(inputs, trace=False, **kw):
    global _compiled
    if _compiled is None:
        _compiled = _build()
    nc = _compiled
    x = np.ascontiguousarray(inputs["x"], dtype=np.float32)
    wq = np.ascontiguousarray(inputs["Wq"], dtype=np.float32)
    wk = np.ascontiguousarray(inputs["Wk"], dtype=np.float32)
    wv = np.ascontiguousarray(inputs["Wv"], dtype=np.float32)
    in_maps = [
        {"x": np.ascontiguousarray(x[i]), "Wq": wq, "Wk": wk, "Wv": wv}
        for i in range(B)
    ]
    res = run_bass_kernel_spmd(nc, in_maps, core_ids=list(range(B)),
                               trace=trace, **kw)
    out = np.stack(
        [np.asarray(res.results[i]["out"]).astype(np.float32) for i in range(B)],
        axis=0)
    return out, res


def kernel(x, Wq, Wk, Wv):
    out, _ = _run({"x": x, "Wq": Wq, "Wk": Wk, "Wv": Wv})
    return out


# revision 39
# speedup vs baseline: 1.1151x; 1.0239x over previous
"""Single-head causal attention on 8 TRN2 NeuronCores.

Problem: x [8, 2048, 1024] f32, Wq/Wk/Wv [1024, 64] f32.
  q = x @ Wq ; k = x @ Wk ; v = x @ Wv        (per batch)
  out = softmax(causal(q k^T / 8)) @ v        [8, 2048, 64]

Sharding: data-parallel over batch -- core i handles batch element i.
No collectives needed.

Per-core kernel (bf16 compute, f32 accumulate), 128-token-tile pipeline:
  1. W loads use the natural row-contiguous layout (2KB descriptors, no
     sub-512B DMA penalty); the d-contraction is chunked INTERLEAVED
     (chunk a = {d : d = 8p + a}) so the natural layout needs no
     on-chip weight transpose -- gpsimd packs [Wq|Wk|Wv] to bf16.
  2. x streams per 256-token group; each 128-tile is cast f32->bf16 on
     DVE directly into the interleaved layout, transposed on the PE
     (8x [128,128] identity matmuls -> PSUM bf16), and copied to the
     x^T SBUF pool by DVE.
  3. Projections are x-stationary: lhsT = x^T tile-chunk, moving
     rhs = [Wq|Wk|Wv] (192 wide) -> PSUM [t,192] in 8 matmuls/tile
     (1536 PE cycles vs 2048 for the W-stationary form).
  4. Q,K land t-major; one DMA-xbar transpose per tile ([t,128] ->
     [qk,t], 8 ucode tiles = ~112ns of DMA) yields Q^T/K^T rows with
     h on partitions. V stays t-major (what PV wants) and is copied
     into V_aug with a ones column (softmax denominator for free).
  5. Attention per 512-row q-block: S^T[tk,tq] = K^T_tile.T @ Q^T
     (contraction h); exp on ACT in k-tile PAIRS (halves the ~185ns
     per-instruction access-latency overhead); causal diagonal via a
     multiplicative 0/1 bf16 mask on DVE; PV accumulates
     out^T[65,tq] += V_aug.T @ P^T in PSUM, row 64 = denominators.
  6. Output: PSUM -> bf16 SBUF copy (gpsimd), PE-transpose back to
     [tq,65], reciprocal-rescale (DVE recip + gpsimd scale), bf16
     store (f32 upcast happens host-side after gather).

Engine budget: PE ~76k cycles (transposes 16.4k, proj 24.6k, S 16.9k,
PV 16.9k, out 1k) is the critical resource; DMA ~29us (x 23.3 =
roofline, W 2.2, qk-xbar 1.8, stores 1.5); ACT owns exp (~20us); DVE
casts/copies/masks (~22us); gpsimd does the PSUM drains (~11us).
"""

import numpy as np

import concourse.bass as bass
import concourse.tile as tile
from concourse import bacc, mybir
from concourse.bass_utils import run_bass_kernel_spmd

B, T, D, H = 8, 2048, 1024, 64
P = 128            # partitions / tile edge
ND = D // P        # 8 d-chunks (interleaved: chunk a = {d : d = 8p + a})
NT = T // P        # 16 token tiles
NB = T // 512      # 4 q-blocks of 512 rows
VA = 80            # v_aug padded k-tile stride

FP32 = mybir.dt.float32
BF16 = mybir.dt.bfloat16

_compiled = None


def _build():
    nc = bacc.Bacc("TRN2", target_bir_lowering=False, debug=False, num_devices=8)

    x_d = nc.dram_tensor("x", [T, D], FP32, kind="ExternalInput").ap()
    wq_d = nc.dram_tensor("Wq", [D, H], FP32, kind="ExternalInput").ap()
    wk_d = nc.dram_tensor("Wk", [D, H], FP32, kind="ExternalInput").ap()
    wv_d = nc.dram_tensor("Wv", [D, H], FP32, kind="ExternalInput").ap()
    out_d = nc.dram_tensor("out", [T, H], BF16, kind="ExternalOutput").ap()

    with tile.TileContext(nc) as tc:
        _kernel(tc, out_d, x_d, wq_d, wk_d, wv_d)

    nc.compile()
    return nc


def _kernel(tc, out_d, x_d, wq_d, wk_d, wv_d):
    nc = tc.nc
    from contextlib import ExitStack

    ctx = ExitStack()
    with ctx:
        const = ctx.enter_context(tc.tile_pool(name="const", bufs=1))
        wstage = ctx.enter_context(tc.tile_pool(name="wstage", bufs=3))
        xload = ctx.enter_context(tc.tile_pool(name="xload", bufs=6))
        xbtp = ctx.enter_context(tc.tile_pool(name="xbtp", bufs=4))
        xtp = ctx.enter_context(tc.tile_pool(name="xtp", bufs=1))
        qkp = ctx.enter_context(tc.tile_pool(name="qkp", bufs=1))
        qksp = ctx.enter_context(tc.tile_pool(name="qksp", bufs=2))
        vsb = ctx.enter_context(tc.tile_pool(name="vsb", bufs=1))
        ptp = ctx.enter_context(tc.tile_pool(name="ptp", bufs=3))
        obp = ctx.enter_context(tc.tile_pool(name="obp", bufs=2))
        osbp = ctx.enter_context(tc.tile_pool(name="osbp", bufs=2))
        recp = ctx.enter_context(tc.tile_pool(name="recp", bufs=2))
        pxt = ctx.enter_context(tc.tile_pool(name="pxt", bufs=1, space="PSUM"))
        psS = ctx.enter_context(tc.tile_pool(name="psS", bufs=2, space="PSUM"))
        pout = ctx.enter_context(tc.tile_pool(name="pout", bufs=1, space="PSUM"))
        psmall = ctx.enter_context(tc.tile_pool(name="psmall", bufs=2, space="PSUM"))

        # ---- loads: first x group, then weights, then remaining x ----
        # (x group 0 first so the cast/transpose pipeline starts ~2us
        # earlier; W only gates the projections, which queue behind.)
        x_r = x_d.rearrange("(g u p) d -> g p u d", p=P, u=2)
        xfs = {}

        def load_x(g):
            xf = xload.tile([P, 2, D], FP32, tag="xf", name=f"xf{g}")
            nc.sync.dma_start(out=xf[:], in_=x_r[g])
            xfs[g] = xf

        load_x(0)

        # Weight loads use the natural row-contiguous layout (2KB
        # descriptors; no sub-512B DMA penalty); chunk a of the interleaved
        # contraction is the partition-slice [:, a, :], so no weight
        # transpose is ever needed.
        w_all = const.tile([P, ND, 3 * H], BF16)   # [Wq | Wk | Wv] per slot
        wnats = []
        for w_dram, name in ((wq_d, "wq"), (wk_d, "wk"), (wv_d, "wv")):
            wn = wstage.tile([P, ND, H], FP32, tag="wstage", name=f"stg_{name}")
            nc.sync.dma_start(out=wn[:], in_=w_dram.rearrange(
                "(p a) h -> p a h", p=P))
            wnats.append(wn)
        for g in range(1, NT // 2):
            load_x(g)

        # Weight pack on ACT: its queue is empty until the first exps
        # (~12us), so waiting on the W loads here blocks nothing.
        for j, wn in enumerate(wnats):
            nc.scalar.copy(out=w_all[:, :, j * H:(j + 1) * H], in_=wn[:])

        # ---- constants ----
        ident_bf = const.tile([P, P], BF16)
        from concourse.masks import make_identity
        make_identity(nc, ident_bf[:])

        # 0/1 upper-triangular (incl. diagonal) bf16 mask in [tk, tq]
        # orientation: valid when tq >= tk (col >= row).
        tri01 = const.tile([P, P], BF16)
        nc.gpsimd.memset(tri01[:], 1.0)
        nc.gpsimd.affine_select(
            out=tri01[:], in_=tri01[:],
            compare_op=mybir.AluOpType.is_ge,
            fill=0.0, base=0,
            pattern=[[1, P]], channel_multiplier=-1)

        # Per-tile projection results, persistent: [:, i, 0:64] = Q,
        # [64:128] = K (both t-major, transposed later on the PE),
        # [128:192] = V, col 192 = 1.0 (softmax-denominator row for the
        # PV matmul's 65-row lhsT).  One DVE copy drains all three.
        vqk = vsb.tile([P, NT, 200], BF16)
        nc.gpsimd.memset(vqk[:, :, 3 * H:3 * H + 1], 1.0)


        # persistent SBUF state
        xT = xtp.tile([P, ND, T], BF16)      # x^T, interleaved chunks
        # Q^T/K^T ([:, 0] = Q^T, [:, 1] = K^T), h on partitions 0:64 --
        # produced by per-tile PE transposes (the DMA-xbar alternative
        # serializes against the x stream on the shared DMA engines and
        # poisons the in-order HWDGE queues with its waits).
        qkT = qkp.tile([H, 2, T], BF16)

        # ---- per-tile pipeline, split front/back and emitted with a
        # one-tile skew (front(i+1) before back(i)) so the PE round-trips
        # inside a tile (transpose->copy->proj->drain->qk-transpose) never
        # expose their latency on the in-order DVE queue.
        def tile_front(i):
            g, u = divmod(i, 2)
            # cast f32 -> bf16 into the interleaved (a, j) layout:
            # element d of the tile lands at [a = d % 8, j = d // 8].
            # Every third tile casts on the (otherwise idle) gpsimd so the
            # DVE keeps up with its PSUM-drain copies.
            xbt = xbtp.tile([P, ND, P], BF16, tag="xbt", name=f"xbt{i}")
            cast_eng = nc.gpsimd if i % 3 == 2 else nc.vector
            cast_eng.tensor_copy(
                out=xbt[:].rearrange("p a j -> p j a"), in_=xfs[g][:, u, :])
            # PE transposes: chunk a -> x^T[:, a, tile i]
            px = pxt.tile([P, ND, P], BF16, tag="pxt", name=f"px{i}")
            for a in range(ND):
                nc.tensor.transpose(px[:, a, :], xbt[:, a, :], ident_bf[:])
            nc.vector.tensor_copy(
                out=xT[:, :, i * P:(i + 1) * P], in_=px[:])

        def tile_back(i):
            # projection: x^T-stationary, W moving (192 wide)
            ps_p = psmall.tile([P, 3 * H], FP32, tag="small", name=f"psp{i}")
            for a in range(ND):
                nc.tensor.matmul(ps_p[:], xT[:, a, i * P:(i + 1) * P],
                                 w_all[:, a, :],
                                 start=(a == 0), stop=(a == ND - 1))
            # single drain: Q|K|V -> vqk (bf16)
            nc.vector.tensor_copy(out=vqk[:, i, 0:3 * H], in_=ps_p[:])
            # Q,K -> PE transposes -> qkT
            pqk = psmall.tile([H, 2, P], BF16, tag="small", name=f"pqk{i}")
            for u in range(2):
                nc.tensor.transpose(pqk[:, u, :], vqk[:, i, u * H:(u + 1) * H],
                                    ident_bf[:])
            nc.vector.tensor_copy(out=qkT[:, :, i * P:(i + 1) * P], in_=pqk[:])

        # ---- attention ----
        stores = []

        def diag(b, ki):
            return 4 * b <= ki < 4 * b + 4

        class AttnBlock:
            """Attention for one 512-row q-block, emitted in STEPS so the
            exp-gated PV matmuls interleave with later tiles' PE work
            instead of head-of-line blocking the in-order PE queue."""

            def __init__(self, b):
                self.b = b
                self.qlo = 512 * b
                self.pairs = [(2 * j, 2 * j + 1) for j in range(2 * b + 2)]
                self.ps_o = pout.tile([H + 1, 512], FP32, tag="pout",
                                      name=f"pso{b}")
                self.idx = 0
                self.pending = self.s_exp(self.pairs[0])
                self.ob = None

            def s_exp(self, pr):
                b, qlo, ps_o = self.b, self.qlo, self.ps_o
                k0, k1 = pr
                w0 = max(0, k0 * P - qlo)
                w1 = max(0, k1 * P - qlo)
                ps = psS.tile([P, 1024], FP32, tag="psS", name=f"psS{b}_{k0}",
                              uniquify=True)
                pt = ptp.tile([P, 1024], BF16, tag="pt", name=f"pt{b}_{k0}")
                for ki, w, pos in ((k0, w0, 0), (k1, w1, 512)):
                    nc.tensor.matmul(
                        ps[:, pos + w:pos + 512],
                        qkT[:, 1, ki * P:(ki + 1) * P],
                        qkT[:, 0, qlo + w:qlo + 512],
                        start=True, stop=True)
                if b == 0:
                    # fresh PSUM slots: exp only over written regions
                    for ki, w, pos in ((k0, w0, 0), (k1, w1, 512)):
                        nc.scalar.activation(
                            out=pt[:, pos + w:pos + 512],
                            in_=ps[:, pos + w:pos + 512],
                            func=mybir.ActivationFunctionType.Exp,
                            scale=0.125)
                else:
                    # one wide exp; the [512, 512+w1) gap holds stale
                    # (finite) values from an earlier pair and is never
                    # read by PV.
                    nc.scalar.activation(
                        out=pt[:, w0:1024], in_=ps[:, w0:1024],
                        func=mybir.ActivationFunctionType.Exp,
                        scale=0.125)
                for ki, w, pos in ((k0, w0, 0), (k1, w1, 512)):
                    if diag(b, ki):
                        nc.gpsimd.tensor_mul(pt[:, pos + w:pos + w + P],
                                             pt[:, pos + w:pos + w + P],
                                             tri01[:])
                return pt, w0, w1

            def done(self):
                return self.idx >= len(self.pairs)

            def step(self):
                """Emit S/exp of pair idx+1 (pipeline-ahead), then PV of
                pair idx.  On the last step, drain ps_o to bf16 SBUF."""
                b, idx, pairs = self.b, self.idx, self.pairs
                k0, k1 = pairs[idx]
                nxt = (self.s_exp(pairs[idx + 1])
                       if idx + 1 < len(pairs) else None)
                pt, w0, w1 = self.pending
                for ki, w, pos in ((k0, w0, 0), (k1, w1, 512)):
                    nc.tensor.matmul(
                        self.ps_o[:, w:512], vqk[:, ki, 2 * H:3 * H + 1],
                        pt[:, pos + w:pos + 512],
                        start=(idx == 0 and ki == k0),
                        stop=(idx == len(pairs) - 1 and ki == k1))
                self.pending = nxt
                self.idx += 1
                if self.done():
                    self.ob = obp.tile([H + 1, 512], BF16, tag="ob",
                                       name=f"ob{b}")
                    nc.scalar.copy(out=self.ob[:], in_=self.ps_o[:])

        def out_stage(b, ob):
            # Deferred to the tail: these waits must never head-of-line
            # block streaming work.
            pot = psmall.tile([P, 4, VA], BF16, tag="small", name=f"pot{b}")
            for j in range(4):
                nc.tensor.transpose(pot[:, j, 0:H + 1],
                                    ob[:, j * P:(j + 1) * P],
                                    ident_bf[0:H + 1, 0:H + 1])
            # Only TWO DVE queue entries here (copy + recip) -- they park in
            # the 4-deep wait queue without stalling DVE.SEQ for the
            # streaming casts behind them; the rescales run on idle gpsimd
            # from SBUF.
            ot = osbp.tile([P, 4, H + 1], BF16, tag="ot", name=f"ot{b}")
            nc.vector.tensor_copy(out=ot[:], in_=pot[:, :, 0:H + 1])
            rec = recp.tile([P, 4], FP32, tag="rec", name=f"rec{b}")
            nc.vector.reciprocal(rec[:], ot[:, :, H])
            osb = osbp.tile([P, 4, H], BF16, tag="osb", name=f"osb{b}")
            for j in range(4):
                nc.gpsimd.tensor_scalar_mul(osb[:, j, :], ot[:, j, 0:H],
                                            rec[:, j:j + 1])
            stores.append(
                (out_d.rearrange("(b j p) h -> b p j h", p=P, j=4)[b], osb))

        # Emission tracks data arrival: a block starts ~2 tiles after its
        # last q-tile's qkT; pair-steps interleave one-per-tile-slot so
        # the exp-gated PVs never clump ahead of later tiles' PE work;
        # out stages slot in after their block finishes.
        START_AT = {6: 0, 9: 1, 13: 2}
        active = []
        finished = []

        def run_steps(budget):
            n = 0
            while active and n < budget:
                blk = active[0]
                blk.step()
                n += 1
                if blk.done():
                    finished.append(active.pop(0))

        SKEW = 2
        for i in range(NT):
            tile_front(i)
            if i >= SKEW:
                tile_back(i - SKEW)
            if i in START_AT:
                active.append(AttnBlock(START_AT[i]))
            run_steps(1 if i < 12 else 2)
        for i in range(NT - SKEW, NT):
            tile_back(i)
        # Block 3 runs at the end; earlier blocks' out stages are woven
        # between its steps so their PE transposes fill the exp-wait
        # bubbles instead of serializing after everything.
        active.append(AttnBlock(3))
        staged = 0
        while active:
            run_steps(2)
            if staged < len(finished):
                blk = finished[staged]
                out_stage(blk.b, blk.ob)
                staged += 1
        for blk in finished[staged:]:
            out_stage(blk.b, blk.ob)

        for dst, osb in stores:
            nc.sync.dma_start(out=dst, in_=osb[:])


def _run(inputs, trace=False, **kw):
    global _compiled
    if _compiled is None:
        _compiled = _build()
    nc = _compiled
    x = np.ascontiguousarray(inputs["x"], dtype=np.float32)
    wq = np.ascontiguousarray(inputs["Wq"], dtype=np.float32)
    wk = np.ascontiguousarray(inputs["Wk"], dtype=np.float32)
    wv = np.ascontiguousarray(inputs["Wv"], dtype=np.float32)
    in_maps = [
        {"x": np.ascontiguousarray(x[i]), "Wq": wq, "Wk": wk, "Wv": wv}
        for i in range(B)
    ]
    res = run_bass_kernel_spmd(nc, in_maps, core_ids=list(range(B)),
                               trace=trace, **kw)
    out = np.stack(
        [np.asarray(res.results[i]["out"]).astype(np.float32) for i in range(B)],
        axis=0)
    return out, res


def kernel(x, Wq, Wk, Wv):
    out, _ = _run({"x": x, "Wq": Wq, "Wk": Wk, "Wv": Wv})
    return out


# revision 40
# speedup vs baseline: 1.1220x; 1.0062x over previous
"""Single-head causal attention on 8 TRN2 NeuronCores.

Problem: x [8, 2048, 1024] f32, Wq/Wk/Wv [1024, 64] f32.
  q = x @ Wq ; k = x @ Wk ; v = x @ Wv        (per batch)
  out = softmax(causal(q k^T / 8)) @ v        [8, 2048, 64]

Sharding: data-parallel over batch -- core i handles batch element i.
No collectives needed.

Per-core kernel (bf16 compute, f32 accumulate), 128-token-tile pipeline:
  1. W loads use the natural row-contiguous layout (2KB descriptors, no
     sub-512B DMA penalty); the d-contraction is chunked INTERLEAVED
     (chunk a = {d : d = 8p + a}) so the natural layout needs no
     on-chip weight transpose -- gpsimd packs [Wq|Wk|Wv] to bf16.
  2. x streams per 256-token group; each 128-tile is cast f32->bf16 on
     DVE directly into the interleaved layout, transposed on the PE
     (8x [128,128] identity matmuls -> PSUM bf16), and copied to the
     x^T SBUF pool by DVE.
  3. Projections are x-stationary: lhsT = x^T tile-chunk, moving
     rhs = [Wq|Wk|Wv] (192 wide) -> PSUM [t,192] in 8 matmuls/tile
     (1536 PE cycles vs 2048 for the W-stationary form).
  4. Q,K land t-major; one DMA-xbar transpose per tile ([t,128] ->
     [qk,t], 8 ucode tiles = ~112ns of DMA) yields Q^T/K^T rows with
     h on partitions. V stays t-major (what PV wants) and is copied
     into V_aug with a ones column (softmax denominator for free).
  5. Attention per 512-row q-block: S^T[tk,tq] = K^T_tile.T @ Q^T
     (contraction h); exp on ACT in k-tile PAIRS (halves the ~185ns
     per-instruction access-latency overhead); causal diagonal via a
     multiplicative 0/1 bf16 mask on DVE; PV accumulates
     out^T[65,tq] += V_aug.T @ P^T in PSUM, row 64 = denominators.
  6. Output: PSUM -> bf16 SBUF copy (gpsimd), PE-transpose back to
     [tq,65], reciprocal-rescale (DVE recip + gpsimd scale), bf16
     store (f32 upcast happens host-side after gather).

Engine budget: PE ~76k cycles (transposes 16.4k, proj 24.6k, S 16.9k,
PV 16.9k, out 1k) is the critical resource; DMA ~29us (x 23.3 =
roofline, W 2.2, qk-xbar 1.8, stores 1.5); ACT owns exp (~20us); DVE
casts/copies/masks (~22us); gpsimd does the PSUM drains (~11us).
"""

import numpy as np

import concourse.bass as bass
import concourse.tile as tile
from concourse import bacc, mybir
from concourse.bass_utils import run_bass_kernel_spmd

B, T, D, H = 8, 2048, 1024, 64
P = 128            # partitions / tile edge
ND = D // P        # 8 d-chunks (interleaved: chunk a = {d : d = 8p + a})
NT = T // P        # 16 token tiles
NB = T // 512      # 4 q-blocks of 512 rows
VA = 80            # v_aug padded k-tile stride

FP32 = mybir.dt.float32
BF16 = mybir.dt.bfloat16

_compiled = None


def _build():
    nc = bacc.Bacc("TRN2", target_bir_lowering=False, debug=False, num_devices=8)

    x_d = nc.dram_tensor("x", [T, D], FP32, kind="ExternalInput").ap()
    wq_d = nc.dram_tensor("Wq", [D, H], FP32, kind="ExternalInput").ap()
    wk_d = nc.dram_tensor("Wk", [D, H], FP32, kind="ExternalInput").ap()
    wv_d = nc.dram_tensor("Wv", [D, H], FP32, kind="ExternalInput").ap()
    out_d = nc.dram_tensor("out", [T, H], BF16, kind="ExternalOutput").ap()

    with tile.TileContext(nc) as tc:
        _kernel(tc, out_d, x_d, wq_d, wk_d, wv_d)

    nc.compile()
    return nc


def _kernel(tc, out_d, x_d, wq_d, wk_d, wv_d):
    nc = tc.nc
    from contextlib import ExitStack

    ctx = ExitStack()
    with ctx:
        const = ctx.enter_context(tc.tile_pool(name="const", bufs=1))
        wstage = ctx.enter_context(tc.tile_pool(name="wstage", bufs=3))
        xload = ctx.enter_context(tc.tile_pool(name="xload", bufs=6))
        xbtp = ctx.enter_context(tc.tile_pool(name="xbtp", bufs=4))
        xtp = ctx.enter_context(tc.tile_pool(name="xtp", bufs=1))
        qkp = ctx.enter_context(tc.tile_pool(name="qkp", bufs=1))
        qksp = ctx.enter_context(tc.tile_pool(name="qksp", bufs=2))
        vsb = ctx.enter_context(tc.tile_pool(name="vsb", bufs=1))
        ptp = ctx.enter_context(tc.tile_pool(name="ptp", bufs=3))
        obp = ctx.enter_context(tc.tile_pool(name="obp", bufs=2))
        osbp = ctx.enter_context(tc.tile_pool(name="osbp", bufs=2))
        recp = ctx.enter_context(tc.tile_pool(name="recp", bufs=2))
        pxt = ctx.enter_context(tc.tile_pool(name="pxt", bufs=1, space="PSUM"))
        psS = ctx.enter_context(tc.tile_pool(name="psS", bufs=2, space="PSUM"))
        pout = ctx.enter_context(tc.tile_pool(name="pout", bufs=1, space="PSUM"))
        psmall = ctx.enter_context(tc.tile_pool(name="psmall", bufs=2, space="PSUM"))

        # ---- loads: first x group, then weights, then remaining x ----
        # (x group 0 first so the cast/transpose pipeline starts ~2us
        # earlier; W only gates the projections, which queue behind.)
        x_r = x_d.rearrange("(g u p) d -> g p u d", p=P, u=2)
        xfs = {}

        def load_x(g):
            xf = xload.tile([P, 2, D], FP32, tag="xf", name=f"xf{g}")
            nc.sync.dma_start(out=xf[:], in_=x_r[g])
            xfs[g] = xf

        load_x(0)

        # Weight loads use the natural row-contiguous layout (2KB
        # descriptors; no sub-512B DMA penalty); chunk a of the interleaved
        # contraction is the partition-slice [:, a, :], so no weight
        # transpose is ever needed.
        w_all = const.tile([P, ND, 3 * H], BF16)   # [Wq | Wk | Wv] per slot
        wnats = []
        for w_dram, name in ((wq_d, "wq"), (wk_d, "wk"), (wv_d, "wv")):
            wn = wstage.tile([P, ND, H], FP32, tag="wstage", name=f"stg_{name}")
            nc.sync.dma_start(out=wn[:], in_=w_dram.rearrange(
                "(p a) h -> p a h", p=P))
            wnats.append(wn)
        for g in range(1, NT // 2):
            load_x(g)

        # Weight pack on ACT: its queue is empty until the first exps
        # (~12us), so waiting on the W loads here blocks nothing.
        for j, wn in enumerate(wnats):
            nc.scalar.copy(out=w_all[:, :, j * H:(j + 1) * H], in_=wn[:])

        # ---- constants ----
        ident_bf = const.tile([P, P], BF16)
        from concourse.masks import make_identity
        make_identity(nc, ident_bf[:])

        # 0/1 upper-triangular (incl. diagonal) bf16 mask in [tk, tq]
        # orientation: valid when tq >= tk (col >= row).
        tri01 = const.tile([P, P], BF16)
        nc.gpsimd.memset(tri01[:], 1.0)
        nc.gpsimd.affine_select(
            out=tri01[:], in_=tri01[:],
            compare_op=mybir.AluOpType.is_ge,
            fill=0.0, base=0,
            pattern=[[1, P]], channel_multiplier=-1)

        # Per-tile projection results, persistent: [:, i, 0:64] = Q,
        # [64:128] = K (both t-major, transposed later on the PE),
        # [128:192] = V, col 192 = 1.0 (softmax-denominator row for the
        # PV matmul's 65-row lhsT).  One DVE copy drains all three.
        vqk = vsb.tile([P, NT, 200], BF16)
        nc.gpsimd.memset(vqk[:, :, 3 * H:3 * H + 1], 1.0)


        # persistent SBUF state
        xT = xtp.tile([P, ND, T], BF16)      # x^T, interleaved chunks
        # Q^T/K^T ([:, 0] = Q^T, [:, 1] = K^T), h on partitions 0:64 --
        # produced by per-tile PE transposes (the DMA-xbar alternative
        # serializes against the x stream on the shared DMA engines and
        # poisons the in-order HWDGE queues with its waits).
        qkT = qkp.tile([H, 2, T], BF16)

        # ---- per-tile pipeline, split front/back and emitted with a
        # one-tile skew (front(i+1) before back(i)) so the PE round-trips
        # inside a tile (transpose->copy->proj->drain->qk-transpose) never
        # expose their latency on the in-order DVE queue.
        def tile_front(i):
            g, u = divmod(i, 2)
            # cast f32 -> bf16 into the interleaved (a, j) layout:
            # element d of the tile lands at [a = d % 8, j = d // 8].
            # Every third tile casts on the (otherwise idle) gpsimd so the
            # DVE keeps up with its PSUM-drain copies.
            xbt = xbtp.tile([P, ND, P], BF16, tag="xbt", name=f"xbt{i}")
            cast_eng = nc.gpsimd if i % 3 == 2 else nc.vector
            cast_eng.tensor_copy(
                out=xbt[:].rearrange("p a j -> p j a"), in_=xfs[g][:, u, :])
            # PE transposes: chunk a -> x^T[:, a, tile i]
            px = pxt.tile([P, ND, P], BF16, tag="pxt", name=f"px{i}")
            for a in range(ND):
                nc.tensor.transpose(px[:, a, :], xbt[:, a, :], ident_bf[:])
            nc.vector.tensor_copy(
                out=xT[:, :, i * P:(i + 1) * P], in_=px[:])

        def tile_back(i):
            # projection: x^T-stationary, W moving (192 wide)
            ps_p = psmall.tile([P, 3 * H], FP32, tag="small", name=f"psp{i}")
            for a in range(ND):
                nc.tensor.matmul(ps_p[:], xT[:, a, i * P:(i + 1) * P],
                                 w_all[:, a, :],
                                 start=(a == 0), stop=(a == ND - 1))
            # single drain: Q|K|V -> vqk (bf16)
            nc.vector.tensor_copy(out=vqk[:, i, 0:3 * H], in_=ps_p[:])
            # Q,K -> PE transposes -> qkT
            pqk = psmall.tile([H, 2, P], BF16, tag="small", name=f"pqk{i}")
            for u in range(2):
                nc.tensor.transpose(pqk[:, u, :], vqk[:, i, u * H:(u + 1) * H],
                                    ident_bf[:])
            nc.vector.tensor_copy(out=qkT[:, :, i * P:(i + 1) * P], in_=pqk[:])

        # ---- attention ----
        stores = []

        def diag(b, ki):
            return 4 * b <= ki < 4 * b + 4

        class AttnBlock:
            """Attention for one 512-row q-block, emitted in STEPS so the
            exp-gated PV matmuls interleave with later tiles' PE work
            instead of head-of-line blocking the in-order PE queue."""

            def __init__(self, b):
                self.b = b
                self.qlo = 512 * b
                self.pairs = [(2 * j, 2 * j + 1) for j in range(2 * b + 2)]
                self.ps_o = pout.tile([H + 1, 512], FP32, tag="pout",
                                      name=f"pso{b}")
                self.idx = 0
                self.pending = self.s_exp(self.pairs[0])
                self.ob = None

            def s_exp(self, pr):
                b, qlo, ps_o = self.b, self.qlo, self.ps_o
                k0, k1 = pr
                w0 = max(0, k0 * P - qlo)
                w1 = max(0, k1 * P - qlo)
                ps = psS.tile([P, 1024], FP32, tag="psS", name=f"psS{b}_{k0}",
                              uniquify=True)
                pt = ptp.tile([P, 1024], BF16, tag="pt", name=f"pt{b}_{k0}")
                for ki, w, pos in ((k0, w0, 0), (k1, w1, 512)):
                    nc.tensor.matmul(
                        ps[:, pos + w:pos + 512],
                        qkT[:, 1, ki * P:(ki + 1) * P],
                        qkT[:, 0, qlo + w:qlo + 512],
                        start=True, stop=True)
                if b == 0:
                    # fresh PSUM slots: exp only over written regions
                    for ki, w, pos in ((k0, w0, 0), (k1, w1, 512)):
                        nc.scalar.activation(
                            out=pt[:, pos + w:pos + 512],
                            in_=ps[:, pos + w:pos + 512],
                            func=mybir.ActivationFunctionType.Exp,
                            scale=0.125)
                else:
                    # one wide exp; the [512, 512+w1) gap holds stale
                    # (finite) values from an earlier pair and is never
                    # read by PV.
                    nc.scalar.activation(
                        out=pt[:, w0:1024], in_=ps[:, w0:1024],
                        func=mybir.ActivationFunctionType.Exp,
                        scale=0.125)
                for ki, w, pos in ((k0, w0, 0), (k1, w1, 512)):
                    if diag(b, ki):
                        nc.gpsimd.tensor_mul(pt[:, pos + w:pos + w + P],
                                             pt[:, pos + w:pos + w + P],
                                             tri01[:])
                return pt, w0, w1

            def done(self):
                return self.idx >= len(self.pairs)

            def step(self):
                """Emit S/exp of pair idx+1 (pipeline-ahead), then PV of
                pair idx.  On the last step, drain ps_o to bf16 SBUF."""
                b, idx, pairs = self.b, self.idx, self.pairs
                k0, k1 = pairs[idx]
                nxt = (self.s_exp(pairs[idx + 1])
                       if idx + 1 < len(pairs) else None)
                pt, w0, w1 = self.pending
                for ki, w, pos in ((k0, w0, 0), (k1, w1, 512)):
                    nc.tensor.matmul(
                        self.ps_o[:, w:512], vqk[:, ki, 2 * H:3 * H + 1],
                        pt[:, pos + w:pos + 512],
                        start=(idx == 0 and ki == k0),
                        stop=(idx == len(pairs) - 1 and ki == k1))
                self.pending = nxt
                self.idx += 1
                if self.done():
                    self.ob = obp.tile([H + 1, 512], BF16, tag="ob",
                                       name=f"ob{b}")
                    nc.scalar.copy(out=self.ob[:], in_=self.ps_o[:])

        def out_stage(b, ob):
            # Deferred to the tail: these waits must never head-of-line
            # block streaming work.
            pot = psmall.tile([P, 4, VA], BF16, tag="small", name=f"pot{b}")
            for j in range(4):
                nc.tensor.transpose(pot[:, j, 0:H + 1],
                                    ob[:, j * P:(j + 1) * P],
                                    ident_bf[0:H + 1, 0:H + 1])
            # Only TWO DVE queue entries here (copy + recip) -- they park in
            # the 4-deep wait queue without stalling DVE.SEQ for the
            # streaming casts behind them; the rescales run on idle gpsimd
            # from SBUF.
            ot = osbp.tile([P, 4, H + 1], BF16, tag="ot", name=f"ot{b}")
            nc.vector.tensor_copy(out=ot[:], in_=pot[:, :, 0:H + 1])
            rec = recp.tile([P, 4], FP32, tag="rec", name=f"rec{b}")
            nc.vector.reciprocal(rec[:], ot[:, :, H])
            osb = osbp.tile([P, 4, H], BF16, tag="osb", name=f"osb{b}")
            for j in range(4):
                nc.gpsimd.tensor_scalar_mul(osb[:, j, :], ot[:, j, 0:H],
                                            rec[:, j:j + 1])
            stores.append(
                (out_d.rearrange("(b j p) h -> b p j h", p=P, j=4)[b], osb))

        # Emission tracks data arrival: a block starts ~2 tiles after its
        # last q-tile's qkT; pair-steps interleave one-per-tile-slot so
        # the exp-gated PVs never clump ahead of later tiles' PE work;
        # out stages slot in after their block finishes.
        START_AT = {6: 0, 9: 1, 13: 2}
        active = []
        finished = []

        def run_steps(budget):
            n = 0
            while active and n < budget:
                blk = active[0]
                blk.step()
                n += 1
                if blk.done():
                    finished.append(active.pop(0))

        SKEW = 2
        for i in range(NT):
            # back(i-2) first: its proj is ready before front(i)'s
            # transposes (which wait on the cast and can overflow the
            # 4-deep PE wait queue, blocking the ready proj behind them).
            if i >= SKEW:
                tile_back(i - SKEW)
            tile_front(i)
            if i in START_AT:
                active.append(AttnBlock(START_AT[i]))
            run_steps(1 if i < 12 else 2)
        for i in range(NT - SKEW, NT):
            tile_back(i)
        # Block 3 runs at the end; earlier blocks' out stages are woven
        # between its steps so their PE transposes fill the exp-wait
        # bubbles instead of serializing after everything.
        active.append(AttnBlock(3))
        staged = 0
        while active:
            run_steps(2)
            if staged < len(finished):
                blk = finished[staged]
                out_stage(blk.b, blk.ob)
                staged += 1
        for blk in finished[staged:]:
            out_stage(blk.b, blk.ob)

        for dst, osb in stores:
            nc.sync.dma_start(out=dst, in_=osb[:])


def _run(inputs, trace=False, **kw):
    global _compiled
    if _compiled is None:
        _compiled = _build()
    nc = _compiled
    x = np.ascontiguousarray(inputs["x"], dtype=np.float32)
    wq = np.ascontiguousarray(inputs["Wq"], dtype=np.float32)
    wk = np.ascontiguousarray(inputs["Wk"], dtype=np.float32)
    wv = np.ascontiguousarray(inputs["Wv"], dtype=np.float32)
    in_maps = [
        {"x": np.ascontiguousarray(x[i]), "Wq": wq, "Wk": wk, "Wv": wv}
        for i in range(B)
    ]
    res = run_bass_kernel_spmd(nc, in_maps, core_ids=list(range(B)),
                               trace=trace, **kw)
    out = np.stack(
        [np.asarray(res.results[i]["out"]).astype(np.float32) for i in range(B)],
        axis=0)
    return out, res


def kernel(x, Wq, Wk, Wv):
    out, _ = _run({"x": x, "Wq": Wq, "Wk": Wk, "Wv": Wv})
    return out
